# revision 30
# baseline (speedup 1.0000x reference)
"""Trainium2 Bass kernel for a 2-layer GAT (heads=1) + linear head + softmax.

Strategy (8 NeuronCores, graph/data parallel):
  - Nodes sharded across cores (12500 dst nodes each); edges partitioned by
    destination node so segment softmax / scatter stay local to a core.
  - Per layer, each core computes projected features for its node shard:
    table row = [h' (HID, bf16) | 1.0 | s_hi | s_lo]  (s = h' @ a_src split
    into two bf16 halves for ~f32 precision), then an AllGather replicates
    the full node table to every core's DRAM (halo exchange).
  - Edges are laid out in "slots": 8 slots per group, 16 groups per
    128-slot chunk, 24 chunks per 128-node destination block (384 groups =
    3 "gsum tiles" per block, padded -> identical program on every core).
  - Main loop per layer: one big indirect-DMA gather of [h'|1|s] rows per
    piece, per-edge attention logits e = leaky_relu(s_src + d_dst) and
    ex = exp(e) (no max-shift needed; logits are bounded), then a two-level
    matmul segment-reduction:
      level 1: ex-carrying selector (static 16-label mask) x gathered rows
               -> per-group partial [sum(ex*h) | sum(ex)]
      level 2: is_equal(group-label, node-iota) selector x group partials
               -> per-node [numerator | denominator] accumulated in PSUM.
    Epilogue divides by the denominator (softmax normalization), adds bias,
    applies relu; layer 2 additionally applies the output head + softmax.
"""

import math
import sys

import numpy as np

if "/opt/trn_rl_repo" not in sys.path:
    sys.path.insert(0, "/opt/trn_rl_repo")

import ml_dtypes

BF16 = ml_dtypes.bfloat16


# ---------------------------------------------------------------- config ---
class Cfg:
    def __init__(self, N, E, n_in=256, hid=128, ncls=3, ncores=8,
                 piece_blocks=5, gb=384):
        self.N, self.E = N, E
        self.N_IN, self.HID, self.NCLS = n_in, hid, ncls
        self.NCORES = ncores
        assert N % ncores == 0
        self.V = N // ncores                      # real dst nodes per core
        self.NB = math.ceil(self.V / 128)         # node blocks per core
        self.VPAD = self.NB * 128
        self.GB = gb                              # groups per block (padded)
        assert gb % 16 == 0
        self.CPB = gb // 16                       # chunks per block
        self.G = self.NB * self.GB                # groups per core
        assert self.G % 128 == 0
        self.NT = self.G // 128                   # gsum tiles per core (NB*3)
        self.TPB = self.GB // 128                 # gsum tiles per block
        assert self.GB % 128 == 0
        self.NCHUNK = self.NB * self.CPB
        self.SLOTS = self.NCHUNK * 128
        self.SENT = N                             # sentinel table row index
        self.ROW = hid + 3                        # h | one | s_hi | s_lo
        self.RHS_W = hid + 1                      # matmul rhs width (h | one)
        # pieces: (block_start, nblocks)
        self.pieces = []
        b = 0
        while b < self.NB:
            nb = min(piece_blocks, self.NB - b)
            self.pieces.append((b, nb))
            b += nb
        self.PIECE_BLOCKS = piece_blocks


DEFAULT_CFG = Cfg(N=100000, E=1600000)


# ---------------------------------------------------- host preprocessing ---
def preprocess(cfg, edge_index):
    """Partition edges by destination core and build per-core slot layout.

    Returns per-core dict of int/float index tensors (identical shapes on
    every core so one NEFF serves all 8).
    """
    src = np.concatenate([edge_index[0], np.arange(cfg.N, dtype=np.int32)])
    dst = np.concatenate([edge_index[1], np.arange(cfg.N, dtype=np.int32)])
    order = np.argsort(dst, kind="stable")
    src, dst = src[order].astype(np.int64), dst[order].astype(np.int64)
    core_of = dst // cfg.V
    bounds = np.searchsorted(core_of, np.arange(cfg.NCORES + 1))
    out = []
    for k in range(cfg.NCORES):
        lo, hi = bounds[k], bounds[k + 1]
        es = src[lo:hi]
        ed = dst[lo:hi] - k * cfg.V               # local dst, sorted
        deg = np.bincount(ed, minlength=cfg.V).astype(np.int64)
        estart = np.zeros(cfg.V + 1, np.int64)
        np.cumsum(deg, out=estart[1:])
        ngrp = (deg + 7) // 8                     # >=1 (self loops)

        src_slot = np.full((128, cfg.NCHUNK), cfg.SENT, np.int32)
        nog = np.full(cfg.G, cfg.VPAD - 1, np.int64)   # node of group
        glab = np.full(cfg.G, 1e9, np.float32)         # in-block node label

        # vectorized slot/group layout
        nodes = np.arange(cfg.V, dtype=np.int64)
        blk = nodes // 128
        # group base of each node within its block
        cumg = np.cumsum(ngrp)
        blk_start_node = blk * 128
        cumg_before_block = np.where(blk_start_node > 0,
                                     cumg[blk_start_node - 1], 0)
        gbase_n = (cumg - ngrp) - cumg_before_block
        blk_tot = np.zeros(cfg.NB, np.int64)
        np.add.at(blk_tot, blk, ngrp)
        assert blk_tot.max() <= cfg.GB, (
            f"core {k}: max groups/block {blk_tot.max()} > {cfg.GB}")
        # groups
        grp_node = np.repeat(nodes, ngrp)               # local node per group
        within = np.arange(len(grp_node), dtype=np.int64) - \
            np.repeat(cumg - ngrp, ngrp)                # 0..ngrp-1
        g_global = blk[grp_node] * cfg.GB + gbase_n[grp_node] + within
        nog[g_global] = grp_node
        glab[g_global] = (grp_node % 128).astype(np.float32)
        # edges -> slots
        n_e = ed                                        # local dst per edge
        j_in = np.arange(len(ed), dtype=np.int64) - estart[n_e]
        grel = gbase_n[n_e] + j_in // 8
        lab = grel % 16
        c = grel // 16
        p = lab + 16 * (j_in % 8)
        chunk = blk[n_e] * cfg.CPB + c
        src_slot[p, chunk] = es.astype(np.int32)
        # [p, T] layouts for the device
        nog_pt = nog.reshape(cfg.NT, 128).T.astype(np.int32).copy()
        glab_pt = glab.reshape(cfg.NT, 128).T.astype(np.float32).copy()
        # int16 dma_gather index layout for the per-piece d-gather:
        # call for piece p covers groups [p*GPP, (p+1)*GPP); sequence pos i
        # lives at [i%16, p*GPP//16 + i//16], replicated over 16-part groups
        gpp = cfg.PIECE_BLOCKS * cfg.GB          # groups per full piece
        nog16 = np.zeros((16, cfg.G // 16), np.int16)
        g_all = np.arange(cfg.G, dtype=np.int64)
        call = g_all // gpp
        i_in = g_all % gpp
        nog16[i_in % 16, call * (gpp // 16) + i_in // 16] = \
            nog.astype(np.int16)
        nog16_full = np.tile(nog16, (8, 1))
        out.append({
            "src_slot": src_slot,
            "nog": nog_pt,
            "nog16": nog16_full,
            "glabel": glab_pt,
        })
    return out


# ------------------------------------------------------------ bass build ---
def build_program(cfg):
    import concourse.bass as bass
    import concourse.bacc as bacc
    import concourse.mybir as mybir
    import concourse.tile as tile
    from concourse.bass import IndirectOffsetOnAxis

    dt = mybir.dt
    F32, BF, I32 = dt.float32, dt.bfloat16, dt.int32
    AF = mybir.ActivationFunctionType
    OP = mybir.AluOpType
    HID, ROW, RHSW, NCLS = cfg.HID, cfg.ROW, cfg.RHS_W, cfg.NCLS

    nc = bacc.Bacc("TRN2", target_bir_lowering=False, debug=False,
                   enable_asserts=False, num_devices=cfg.NCORES)

    # ---- I/O ----
    xT = nc.dram_tensor("xT", [cfg.N_IN, cfg.VPAD], F32, kind="ExternalInput")
    W1 = nc.dram_tensor("W1", [cfg.N_IN, HID], F32, kind="ExternalInput")
    W2 = nc.dram_tensor("W2", [HID, HID], F32, kind="ExternalInput")
    Wo = nc.dram_tensor("Wo", [HID, NCLS], F32, kind="ExternalInput")
    a1s = nc.dram_tensor("a1s", [HID], F32, kind="ExternalInput")
    a1d = nc.dram_tensor("a1d", [HID], F32, kind="ExternalInput")
    a2s = nc.dram_tensor("a2s", [HID], F32, kind="ExternalInput")
    a2d = nc.dram_tensor("a2d", [HID], F32, kind="ExternalInput")
    b1 = nc.dram_tensor("b1", [HID], F32, kind="ExternalInput")
    b2 = nc.dram_tensor("b2", [HID], F32, kind="ExternalInput")
    bo = nc.dram_tensor("bo", [NCLS], F32, kind="ExternalInput")
    src_slot = nc.dram_tensor("src_slot", [128, cfg.NCHUNK], I32,
                              kind="ExternalInput")
    nog_in = nc.dram_tensor("nog", [128, cfg.NT], I32, kind="ExternalInput")
    nog16_in = nc.dram_tensor("nog16", [128, cfg.G // 16], mybir.dt.int16,
                              kind="ExternalInput")
    glab_in = nc.dram_tensor("glabel", [128, cfg.NT], F32,
                             kind="ExternalInput")
    out_t = nc.dram_tensor("out", [cfg.VPAD, NCLS], F32,
                           kind="ExternalOutput")

    # ---- inline constants ----
    ident_d = nc.inline_tensor(np.eye(128, dtype=np.float32), "ident")
    mask_np = (np.arange(128)[:, None] % 16 == np.arange(16)[None, :])
    mask_d = nc.inline_tensor(mask_np.astype(BF16), "mask16")
    iota_d = nc.inline_tensor(
        np.tile(np.arange(128, dtype=np.float32), (128, 1)), "iota2d")
    sent_np = np.zeros((1, ROW), BF16)
    sent_np[0, HID + 1] = BF16(-10000.0)
    sent_d = nc.inline_tensor(sent_np, "sentrow")

    groups = [list(range(cfg.NCORES))]

    with tile.TileContext(nc, num_cores=cfg.NCORES) as tc:
        with (
            tc.tile_pool(name="const", bufs=1) as cp,
            tc.tile_pool(name="gath", bufs=2) as gp,
            tc.tile_pool(name="work", bufs=2) as wp,
            tc.tile_pool(name="gsum", bufs=6) as sp,
            tc.tile_pool(name="psum", bufs=2, space="PSUM") as pp,
            tc.tile_pool(name="dram", bufs=1, space="DRAM") as dp,
        ):
            # ======== constants to SBUF ========
            ident = cp.tile([128, 128], F32, tag="ident")
            nc.sync.dma_start(ident[:], ident_d[:, :])
            mask16 = cp.tile([128, 16], BF, tag="mask16")
            nc.sync.dma_start(mask16[:], mask_d[:, :])
            iota2 = cp.tile([128, 128], F32, tag="iota2")
            nc.sync.dma_start(iota2[:], iota_d[:, :])
            srcsb = cp.tile([128, cfg.NCHUNK], I32, tag="srcsb")
            nc.sync.dma_start(srcsb[:], src_slot[:, :])
            nogsb = cp.tile([128, cfg.NT], I32, tag="nogsb")
            nc.sync.dma_start(nogsb[:], nog_in[:, :])
            nog16sb = cp.tile([128, cfg.G // 16], mybir.dt.int16,
                              tag="nog16sb")
            nc.sync.dma_start(nog16sb[:], nog16_in[:, :])
            glabsb = cp.tile([128, cfg.NT], F32, tag="glabsb")
            nc.sync.dma_start(glabsb[:], glab_in[:, :])
            wosb = cp.tile([128, NCLS], F32, tag="wosb")
            nc.sync.dma_start(wosb[:], Wo[:, :])
            b1r = cp.tile([128, HID], F32, tag="b1r")
            nc.sync.dma_start(b1r[:], b1[None, :].to_broadcast([128, HID]))
            b2r = cp.tile([128, HID], F32, tag="b2r")
            nc.sync.dma_start(b2r[:], b2[None, :].to_broadcast([128, HID]))
            bor = cp.tile([128, NCLS], F32, tag="bor")
            nc.sync.dma_start(bor[:], bo[None, :].to_broadcast([128, NCLS]))

            def make_rhs(Wd, asd_s, asd_d, nchunks, tagbase):
                """rhs tiles [128, HID+2] = [W chunk | W@a_src | W@a_dst]."""
                asd = cp.tile([128, 2], F32, tag=tagbase + "_asd")
                nc.sync.dma_start(asd[:, 0:1], asd_s[:, None])
                nc.sync.dma_start(asd[:, 1:2], asd_d[:, None])
                tiles = []
                for c in range(nchunks):
                    rt = cp.tile([128, HID + 2], F32, tag=f"{tagbase}_{c}")
                    nc.sync.dma_start(rt[:, 0:HID],
                                      Wd[c * 128:(c + 1) * 128, :])
                    tp = pp.tile([128, 128], F32, tag="mm")
                    nc.tensor.transpose(tp[:], rt[:, 0:HID], ident[:])
                    wt = wp.tile([128, 128], F32, tag="wt")
                    nc.scalar.activation(wt[:], tp[:], AF.Copy)
                    sp2 = pp.tile([128, 2], F32, tag="mm")
                    nc.tensor.matmul(sp2[:], lhsT=wt[:], rhs=asd[:],
                                     start=True, stop=True)
                    nc.vector.tensor_copy(rt[:, HID:HID + 2], sp2[:])
                    tiles.append(rt)
                return tiles

            import os as _os
            _stage = int(_os.environ.get("GAT_STAGE", "6"))
            if _stage >= 1:
                rhs1 = make_rhs(W1, a1s, a1d, cfg.N_IN // 128, "rhs1")
                rhs2 = make_rhs(W2, a2s, a2d, 1, "rhs2")

            h2st = cp.tile([128, cfg.NB * HID], F32, tag="h2st")

            # persistent selector tiles: per chunk-pair layout is
            # [realA(16) | zeros(16) | realB(16)] so that the [*,32] lhsT
            # slice of either chunk has true zeros in its other half.
            # memset once; per-piece builds only touch the real columns.
            SELW = 48 * (cfg.PIECE_BLOCKS * cfg.CPB // 2)
            selA = cp.tile([128, SELW], BF, tag="selA")
            selB = cp.tile([128, SELW], BF, tag="selB")
            nc.gpsimd.memset(selA[:], 0.0)
            nc.gpsimd.memset(selB[:], 0.0)

            # DRAM scratch
            tables = [dp.tile([cfg.N + 1, ROW], BF, name=f"table{i}",
                              tag=f"table{i}") for i in range(2)]
            shards = [dp.tile([cfg.V, ROW], BF, name=f"shard{i}",
                              tag=f"shard{i}") for i in range(2)]
            d_drams = [dp.tile([cfg.VPAD, 1], F32, name=f"ddram{i}",
                               tag=f"ddram{i}") for i in range(2)]
            dgds = [dp.tile([cfg.G], F32, name=f"dgd{i}", tag=f"dgd{i}")
                    for i in range(2)]
            dsds = [dp.tile([cfg.SLOTS], F32, name=f"dsd{i}", tag=f"dsd{i}")
                    for i in range(2)]


            # ======== per-layer table prep ========
            def build_rows(ppre, b, shard, d_dram):
                """ppre: psum [128, HID+2] = [h' | s | d] for block b."""
                rows = wp.tile([128, ROW], BF, tag="rows")
                nc.scalar.activation(rows[:, 0:HID], ppre[:, 0:HID], AF.Copy)
                nc.gpsimd.memset(rows[:, HID:HID + 1], 1.0)
                nc.scalar.activation(rows[:, HID + 1:HID + 2],
                                     ppre[:, HID:HID + 1], AF.Copy)
                shi = wp.tile([128, 1], F32, tag="shi")
                nc.vector.tensor_copy(shi[:], rows[:, HID + 1:HID + 2])
                nc.vector.tensor_tensor(rows[:, HID + 2:HID + 3],
                                        ppre[:, HID:HID + 1], shi[:],
                                        op=OP.subtract)
                dcol = wp.tile([128, 1], F32, tag="dcol")
                nc.vector.tensor_copy(dcol[:], ppre[:, HID + 1:HID + 2])
                nc.sync.dma_start(d_dram[b * 128:(b + 1) * 128, :], dcol[:])
                nrow = min(128, cfg.V - b * 128)
                nc.sync.dma_start(shard[b * 128:b * 128 + nrow, :],
                                  rows[0:nrow, :])

            def prep_layer1():
                for b in range(cfg.NB):
                    ppre = pp.tile([128, HID + 2], F32, tag="mm")
                    for c in range(cfg.N_IN // 128):
                        xt = wp.tile([128, 128], F32, tag="xt")
                        nc.sync.dma_start(
                            xt[:],
                            xT[c * 128:(c + 1) * 128,
                               b * 128:(b + 1) * 128])
                        nc.tensor.matmul(ppre[:], lhsT=xt[:], rhs=rhs1[c][:],
                                         start=(c == 0),
                                         stop=(c == cfg.N_IN // 128 - 1))
                    build_rows(ppre, b, shards[0], d_drams[0])

            def prep_layer2():
                for b in range(cfg.NB):
                    tp = pp.tile([128, 128], F32, tag="mm")
                    nc.tensor.transpose(
                        tp[:], h2st[:, b * HID:(b + 1) * HID], ident[:])
                    h2T = wp.tile([128, 128], F32, tag="h2T")
                    nc.scalar.activation(h2T[:], tp[:], AF.Copy)
                    ppre = pp.tile([128, HID + 2], F32, tag="mm")
                    nc.tensor.matmul(ppre[:], lhsT=h2T[:], rhs=rhs2[0][:],
                                     start=True, stop=True)
                    build_rows(ppre, b, shards[1], d_drams[1])

            def allgather(li):
                nc.gpsimd.collective_compute(
                    "AllGather", mybir.AluOpType.bypass,
                    replica_groups=groups,
                    ins=[shards[li][:, :].opt()],
                    outs=[tables[li][0:cfg.N, :].opt()],
                )
                nc.sync.dma_start(tables[li][cfg.N:cfg.N + 1, :],
                                  sent_d[:, :])

            # ======== main per-layer loop ========
            def main_layer(li, epilogue):
                table, d_dram = tables[li], d_drams[li]
                dgd, dsd = dgds[li], dsds[li]
                PB = cfg.PIECE_BLOCKS
                for pi, (b0, nb) in enumerate(cfg.pieces):
                    Tn = nb * cfg.TPB
                    NCh = nb * cfg.CPB
                    Gn = nb * cfg.GB
                    gbase = b0 * cfg.GB
                    # d per group (indirect gather), to DRAM, expand, reload
                    dg = wp.tile([128, PB * cfg.TPB], F32, tag="dg")
                    for t in range(Tn):
                        nc.gpsimd.indirect_dma_start(
                            out=dg[:, t:t + 1], out_offset=None,
                            in_=d_dram[:, :],
                            in_offset=IndirectOffsetOnAxis(
                                ap=nogsb[:, b0 * cfg.TPB + t:
                                         b0 * cfg.TPB + t + 1],
                                axis=0))
                    nc.sync.dma_start(
                        dgd[gbase:gbase + Gn].rearrange("(t p) -> p t", p=128),
                        dg[:, 0:Tn])
                    nc.sync.dma_start(
                        dsd[b0 * cfg.CPB * 128:(b0 * cfg.CPB + NCh) * 128]
                        .rearrange("(bb c i l) -> bb c i l",
                                   bb=nb, c=cfg.CPB, i=8, l=16),
                        dgd[gbase:gbase + Gn]
                        .rearrange("(bb c l) -> bb c () l",
                                   bb=nb, c=cfg.CPB, l=16)
                        .to_broadcast([nb, cfg.CPB, 8, 16]))
                    dsl = wp.tile([128, PB * cfg.CPB], F32, tag="dsl")
                    nc.sync.dma_start(
                        dsl[:, 0:NCh],
                        dsd[b0 * cfg.CPB * 128:(b0 * cfg.CPB + NCh) * 128]
                        .rearrange("(j p) -> p j", p=128))
                    # feature gather: one 128-row indirect DMA per slot-chunk
                    gt = gp.tile([128, ROW * PB * cfg.CPB], BF, tag="gt")
                    for j in range(NCh):
                        nc.gpsimd.indirect_dma_start(
                            out=gt[:, ROW * j:ROW * (j + 1)], out_offset=None,
                            in_=table[:, :],
                            in_offset=IndirectOffsetOnAxis(
                                ap=srcsb[:, b0 * cfg.CPB + j:
                                         b0 * cfg.CPB + j + 1],
                                axis=0))
                    gv = gt[:, 0:ROW * NCh].rearrange(
                        "p (j c) -> p j c", c=ROW)
                    # phase A: ex = exp(leaky_relu(s_src + d_dst))
                    sf = wp.tile([128, PB * cfg.CPB], F32, tag="sf")
                    sfv = sf[:, 0:NCh].rearrange("p j -> p j ()")
                    nc.vector.tensor_tensor(
                        sfv, gv[:, :, HID + 1:HID + 2],
                        gv[:, :, HID + 2:HID + 3], op=OP.add)
                    ep = wp.tile([128, PB * cfg.CPB], F32, tag="ep")
                    nc.vector.tensor_tensor(ep[:, 0:NCh], sf[:, 0:NCh],
                                            dsl[:, 0:NCh], op=OP.add)
                    es = wp.tile([128, PB * cfg.CPB], F32, tag="es")
                    nc.vector.tensor_scalar_mul(es[:, 0:NCh], ep[:, 0:NCh],
                                                0.2)
                    el = wp.tile([128, PB * cfg.CPB], F32, tag="el")
                    nc.vector.tensor_tensor(el[:, 0:NCh], ep[:, 0:NCh],
                                            es[:, 0:NCh], op=OP.max)
                    exf = wp.tile([128, PB * cfg.CPB], F32, tag="exf")
                    nc.scalar.activation(exf[:, 0:NCh], el[:, 0:NCh], AF.Exp)
                    exb = wp.tile([128, PB * cfg.CPB], BF, tag="exb")
                    nc.vector.tensor_copy(exb[:, 0:NCh], exf[:, 0:NCh])
                    # selector build: even chunks -> cols [48a, 48a+16),
                    # odd chunks -> cols [48a+32, 48a+48)
                    sel1 = selA if (b0 // cfg.PIECE_BLOCKS) % 2 == 0 else selB
                    npair = NCh // 2
                    exv = exb[:, 0:NCh].rearrange("p (a u) -> p a u", u=2)
                    maskv = mask16[:].rearrange("p l -> p () l") \
                        .to_broadcast([128, npair, 16])
                    selv = sel1[:, 0:48 * npair].rearrange(
                        "p (a w) -> p a w", w=48)
                    nc.vector.tensor_tensor(
                        selv[:, :, 0:16],
                        exv[:, :, 0:1].to_broadcast([128, npair, 16]),
                        maskv, op=OP.mult)
                    nc.vector.tensor_tensor(
                        selv[:, :, 32:48],
                        exv[:, :, 1:2].to_broadcast([128, npair, 16]),
                        maskv, op=OP.mult)
                    # level 1 + level 2
                    for bb in range(nb):
                        b = b0 + bb
                        pl2 = pp.tile([128, RHSW], F32, tag="l2")
                        for t in range(cfg.TPB):
                            pl1 = pp.tile([128, RHSW], F32, tag="l1")
                            for al in range(4):
                                for u in range(2):
                                    j = bb * cfg.CPB + t * 8 + 2 * al + u
                                    A = j // 2
                                    nc.tensor.matmul(
                                        pl1[32 * al:32 * al + 32, :],
                                        lhsT=sel1[:, 48 * A + 16 * u:
                                                  48 * A + 16 * u + 32],
                                        rhs=gt[:, ROW * j:ROW * j + RHSW],
                                        start=(u == 0), stop=(u == 1),
                                        tile_position=(0, 32 * al))
                            gs = sp.tile([128, RHSW], F32, tag="gsum")
                            nc.scalar.activation(gs[:], pl1[:], AF.Copy)
                            T = b * cfg.TPB + t
                            l2s = wp.tile([128, 128], F32, tag="l2s")
                            nc.vector.tensor_tensor(
                                l2s[:],
                                glabsb[:, T:T + 1].to_broadcast([128, 128]),
                                iota2[:], op=OP.is_equal)
                            nc.tensor.matmul(pl2[:], lhsT=l2s[:], rhs=gs[:],
                                             start=(t == 0),
                                             stop=(t == cfg.TPB - 1))
                        epilogue(b, pl2)

            def epi_norm(pl2, brow):
                den = wp.tile([128, 1], F32, tag="den")
                nc.vector.tensor_scalar_max(den[:], pl2[:, HID:HID + 1],
                                            1e-30)
                rec = wp.tile([128, 1], F32, tag="rec")
                nc.vector.reciprocal(rec[:], den[:])
                hb = wp.tile([128, HID], F32, tag="hb")
                nc.vector.tensor_scalar_mul(hb[:], pl2[:, 0:HID],
                                            rec[:, 0:1])
                nc.vector.tensor_tensor(hb[:], hb[:], brow[:], op=OP.add)
                return hb

            def epilogue1(b, pl2):
                hb = epi_norm(pl2, b1r)
                nc.scalar.activation(h2st[:, b * HID:(b + 1) * HID], hb[:],
                                     AF.Relu)

            def epilogue2(b, pl2):
                hb = epi_norm(pl2, b2r)
                h3 = wp.tile([128, HID], F32, tag="h3")
                nc.scalar.activation(h3[:], hb[:], AF.Relu)
                tp = pp.tile([128, 128], F32, tag="mm")
                nc.tensor.transpose(tp[:], h3[:], ident[:])
                h3T = wp.tile([128, 128], F32, tag="h3T")
                nc.scalar.activation(h3T[:], tp[:], AF.Copy)
                po = pp.tile([128, NCLS], F32, tag="mm")
                nc.tensor.matmul(po[:], lhsT=h3T[:], rhs=wosb[:],
                                 start=True, stop=True)
                lg = wp.tile([128, NCLS], F32, tag="lg")
                nc.vector.tensor_tensor(lg[:], po[:], bor[:], op=OP.add)
                mx = wp.tile([128, 1], F32, tag="mx")
                nc.vector.tensor_reduce(mx[:], lg[:],
                                        axis=mybir.AxisListType.X, op=OP.max)
                lgs = wp.tile([128, NCLS], F32, tag="lgs")
                nc.vector.tensor_scalar(lgs[:], lg[:], mx[:, 0:1], None,
                                        op0=OP.subtract)
                pe = wp.tile([128, NCLS], F32, tag="pe")
                rs = wp.tile([128, 1], F32, tag="rs")
                nc.scalar.activation(pe[:], lgs[:], AF.Exp,
                                     accum_out=rs[:, 0:1])
                rr = wp.tile([128, 1], F32, tag="rr")
                nc.vector.reciprocal(rr[:], rs[:])
                ot = wp.tile([128, NCLS], F32, tag="ot")
                nc.vector.tensor_scalar_mul(ot[:], pe[:], rr[:, 0:1])
                nc.sync.dma_start(out_t[b * 128:(b + 1) * 128, :], ot[:])

            # ======== schedule ========
            stage = _stage
            if stage >= 2:
                prep_layer1()
            if stage >= 3:
                allgather(0)
            if stage >= 4:
                main_layer(0, epilogue1)
            if stage >= 5:
                prep_layer2()
                allgather(1)
            if stage >= 6:
                main_layer(1, epilogue2)
            if stage < 6:
                # touch the output so the program remains well formed
                zz = wp.tile([128, NCLS], F32, tag="zz")
                nc.gpsimd.memset(zz[:], 0.0)
                for b in range(cfg.NB):
                    nc.sync.dma_start(out_t[b * 128:(b + 1) * 128, :], zz[:])

    nc._gat_dbg = {
        "table0": tables[0].tensor.name, "table1": tables[1].tensor.name,
        "shard0": shards[0].tensor.name, "shard1": shards[1].tensor.name,
        "ddram0": d_drams[0].tensor.name, "ddram1": d_drams[1].tensor.name,
        "dgd0": dgds[0].tensor.name, "dsd0": dsds[0].tensor.name,
    }
    nc.compile()
    return nc


# ------------------------------------------------------------------ run ---
_PROG_CACHE = {}


def _get_program(cfg):
    key = (cfg.N, cfg.E, cfg.NCORES)
    if key not in _PROG_CACHE:
        _PROG_CACHE[key] = build_program(cfg)
    return _PROG_CACHE[key]


_PRE_CACHE = {}


def run(cfg, inputs, trace=False):
    from concourse.bass_utils import run_bass_kernel_spmd

    nc = _get_program(cfg)
    ei = np.asarray(inputs["edge_index"])
    pkey = (cfg.N, cfg.E, int(ei[0, :16].sum()), int(ei[1, -16:].sum()))
    if pkey not in _PRE_CACHE:
        _PRE_CACHE[pkey] = preprocess(cfg, ei)
    pre = _PRE_CACHE[pkey]
    in_maps = _make_in_maps(cfg, inputs, pre)
    res = run_bass_kernel_spmd(nc, in_maps, core_ids=list(range(cfg.NCORES)),
                               trace=trace)
    outs = [res.results[k]["out"][:cfg.V] for k in range(cfg.NCORES)]
    full = np.concatenate(outs, axis=0).astype(np.float32)
    return full, res


def timed_run(cfg, inputs, iters=3):
    """Time device execution with device-resident inputs (axon transfer
    excluded). Returns (best_seconds, outputs_of_last_iter)."""
    import time

    import jax
    from jax.sharding import Mesh, PartitionSpec
    from jax.experimental.shard_map import shard_map
    import concourse.mybir as mybir
    from concourse import bass2jax
    from concourse.bass2jax import _bass_exec_p, partition_id_tensor

    bass2jax.install_neuronx_cc_hook()
    nc = _get_program(cfg)
    ei = np.asarray(inputs["edge_index"])
    pkey = (cfg.N, cfg.E, int(ei[0, :16].sum()), int(ei[1, -16:].sum()))
    if pkey not in _PRE_CACHE:
        _PRE_CACHE[pkey] = preprocess(cfg, ei)
    pre = _PRE_CACHE[pkey]
    in_maps = _make_in_maps(cfg, inputs, pre)

    partition_name = (nc.partition_id_tensor.name
                      if nc.partition_id_tensor else None)
    in_names, out_names, out_avals, zero_outs = [], [], [], []
    for alloc in nc.m.functions[0].allocations:
        if not isinstance(alloc, mybir.MemoryLocationSet):
            continue
        name = alloc.memorylocations[0].name
        if alloc.kind == "ExternalInput":
            if name != partition_name:
                in_names.append(name)
        elif alloc.kind == "ExternalOutput":
            out_names.append(name)
            shape = tuple(alloc.tensor_shape)
            dtype = mybir.dt.np(alloc.dtype)
            out_avals.append(jax.core.ShapedArray(shape, dtype))
            zero_outs.append(np.zeros(shape, dtype))
    n_params = len(in_names)
    n_outs = len(out_avals)
    all_in_names = list(in_names) + list(out_names)
    if partition_name is not None:
        all_in_names.append(partition_name)
    donate = tuple(range(n_params, n_params + n_outs))

    def _body(*args):
        operands = list(args)
        if partition_name is not None:
            operands.append(partition_id_tensor())
        outs = _bass_exec_p.bind(
            *operands, out_avals=tuple(out_avals),
            in_names=tuple(all_in_names), out_names=tuple(out_names),
            lowering_input_output_aliases=(),
            sim_require_finite=True, sim_require_nnan=True, nc=nc)
        return tuple(outs)

    devices = jax.devices()[:cfg.NCORES]
    mesh = Mesh(np.asarray(devices), ("core",))
    in_specs = (PartitionSpec("core"),) * (n_params + n_outs)
    out_specs = (PartitionSpec("core"),) * len(out_names)
    sharded = jax.jit(
        shard_map(_body, mesh=mesh, in_specs=in_specs, out_specs=out_specs,
                  check_rep=False),
        donate_argnums=donate, keep_unused=True)
    concat_in = [
        np.concatenate([np.asarray(in_maps[c][nm]) for c in range(cfg.NCORES)],
                       axis=0)
        for nm in in_names]
    sharding = jax.sharding.NamedSharding(mesh, PartitionSpec("core"))
    dev_in = [jax.device_put(a, sharding) for a in concat_in]
    times = []
    out_arrs = None
    for _ in range(iters):
        zo = [jax.device_put(
            np.zeros((cfg.NCORES * z.shape[0], *z.shape[1:]), z.dtype),
            sharding) for z in zero_outs]
        jax.block_until_ready(zo)
        t0 = time.time()
        out_arrs = sharded(*dev_in, *zo)
        jax.block_until_ready(out_arrs)
        times.append(time.time() - t0)
    oi = out_names.index("out")
    full = np.asarray(out_arrs[oi]).reshape(cfg.NCORES, cfg.VPAD, cfg.NCLS)
    out = np.concatenate([full[c, :cfg.V] for c in range(cfg.NCORES)], axis=0)
    return min(times), out.astype(np.float32)


def _make_in_maps(cfg, inputs, pre):
    x = np.asarray(inputs["x"], np.float32)
    common = {
        "W1": np.asarray(inputs["W1"], np.float32),
        "W2": np.asarray(inputs["W2"], np.float32),
        "Wo": np.asarray(inputs["Wo"], np.float32),
        "a1s": np.asarray(inputs["a1_src"], np.float32),
        "a1d": np.asarray(inputs["a1_dst"], np.float32),
        "a2s": np.asarray(inputs["a2_src"], np.float32),
        "a2d": np.asarray(inputs["a2_dst"], np.float32),
        "b1": np.asarray(inputs["b1"], np.float32),
        "b2": np.asarray(inputs["b2"], np.float32),
        "bo": np.asarray(inputs["bo"], np.float32),
    }
    in_maps = []
    for k in range(cfg.NCORES):
        xs = x[k * cfg.V:(k + 1) * cfg.V]
        xT = np.zeros((cfg.N_IN, cfg.VPAD), np.float32)
        xT[:, :cfg.V] = xs.T
        m = dict(common)
        m["xT"] = np.ascontiguousarray(xT)
        m["src_slot"] = pre[k]["src_slot"]
        m["nog"] = pre[k]["nog"]
        m["nog16"] = pre[k]["nog16"]
        m["glabel"] = pre[k]["glabel"]
        in_maps.append(m)
    return in_maps


def kernel(**inputs):
    cfg = DEFAULT_CFG
    full, _ = run(cfg, inputs, trace=False)
    return full


# revision 33
# speedup vs baseline: 1.1871x; 1.1871x over previous
"""Trainium2 Bass kernel for a 2-layer GAT (heads=1) + linear head + softmax.

Strategy (8 NeuronCores, graph/data parallel):
  - Nodes sharded across cores (12500 dst nodes each); edges partitioned by
    destination node so segment softmax / scatter stay local to a core.
  - Per layer, each core computes projected features for its node shard:
    table row = [h' (HID, bf16) | 1.0 | s_hi | s_lo]  (s = h' @ a_src split
    into two bf16 halves for ~f32 precision), then an AllGather replicates
    the full node table to every core's DRAM (halo exchange).
  - Edges are laid out in "slots": 8 slots per group, 16 groups per
    128-slot chunk, 24 chunks per 128-node destination block (384 groups =
    3 "gsum tiles" per block, padded -> identical program on every core).
  - Main loop per layer: one big indirect-DMA gather of [h'|1|s] rows per
    piece, per-edge attention logits e = leaky_relu(s_src + d_dst) and
    ex = exp(e) (no max-shift needed; logits are bounded), then a two-level
    matmul segment-reduction:
      level 1: ex-carrying selector (static 16-label mask) x gathered rows
               -> per-group partial [sum(ex*h) | sum(ex)]
      level 2: is_equal(group-label, node-iota) selector x group partials
               -> per-node [numerator | denominator] accumulated in PSUM.
    Epilogue divides by the denominator (softmax normalization), adds bias,
    applies relu; layer 2 additionally applies the output head + softmax.
"""

import math
import sys

import numpy as np

if "/opt/trn_rl_repo" not in sys.path:
    sys.path.insert(0, "/opt/trn_rl_repo")

import ml_dtypes

BF16 = ml_dtypes.bfloat16


# ---------------------------------------------------------------- config ---
class Cfg:
    def __init__(self, N, E, n_in=256, hid=128, ncls=3, ncores=8,
                 piece_blocks=5, gb=384):
        self.N, self.E = N, E
        self.N_IN, self.HID, self.NCLS = n_in, hid, ncls
        self.NCORES = ncores
        assert N % ncores == 0
        self.V = N // ncores                      # real dst nodes per core
        self.NB = math.ceil(self.V / 128)         # node blocks per core
        self.VPAD = self.NB * 128
        self.GB = gb                              # groups per block (padded)
        assert gb % 16 == 0
        self.CPB = gb // 16                       # chunks per block
        self.G = self.NB * self.GB                # groups per core
        assert self.G % 128 == 0
        self.NT = self.G // 128                   # gsum tiles per core (NB*3)
        self.TPB = self.GB // 128                 # gsum tiles per block
        assert self.GB % 128 == 0
        self.NCHUNK = self.NB * self.CPB
        self.SLOTS = self.NCHUNK * 128
        self.SENT = N                             # sentinel table row index
        self.ROW = hid + 3                        # h | one | s_hi | s_lo
        self.RHS_W = hid + 1                      # matmul rhs width (h | one)
        # pieces: (block_start, nblocks)
        self.pieces = []
        b = 0
        while b < self.NB:
            nb = min(piece_blocks, self.NB - b)
            self.pieces.append((b, nb))
            b += nb
        self.PIECE_BLOCKS = piece_blocks


DEFAULT_CFG = Cfg(N=100000, E=1600000)


# ---------------------------------------------------- host preprocessing ---
def preprocess(cfg, edge_index):
    """Partition edges by destination core and build per-core slot layout.

    Returns per-core dict of int/float index tensors (identical shapes on
    every core so one NEFF serves all 8).
    """
    src = np.concatenate([edge_index[0], np.arange(cfg.N, dtype=np.int32)])
    dst = np.concatenate([edge_index[1], np.arange(cfg.N, dtype=np.int32)])
    order = np.argsort(dst, kind="stable")
    src, dst = src[order].astype(np.int64), dst[order].astype(np.int64)
    core_of = dst // cfg.V
    bounds = np.searchsorted(core_of, np.arange(cfg.NCORES + 1))
    out = []
    for k in range(cfg.NCORES):
        lo, hi = bounds[k], bounds[k + 1]
        es = src[lo:hi]
        ed = dst[lo:hi] - k * cfg.V               # local dst, sorted
        deg = np.bincount(ed, minlength=cfg.V).astype(np.int64)
        estart = np.zeros(cfg.V + 1, np.int64)
        np.cumsum(deg, out=estart[1:])
        ngrp = (deg + 7) // 8                     # >=1 (self loops)

        src_slot = np.full((128, cfg.NCHUNK), cfg.SENT, np.int32)
        nog = np.full(cfg.G, cfg.VPAD - 1, np.int64)   # node of group
        glab = np.full(cfg.G, 1e9, np.float32)         # in-block node label

        # vectorized slot/group layout
        nodes = np.arange(cfg.V, dtype=np.int64)
        blk = nodes // 128
        # group base of each node within its block
        cumg = np.cumsum(ngrp)
        blk_start_node = blk * 128
        cumg_before_block = np.where(blk_start_node > 0,
                                     cumg[blk_start_node - 1], 0)
        gbase_n = (cumg - ngrp) - cumg_before_block
        blk_tot = np.zeros(cfg.NB, np.int64)
        np.add.at(blk_tot, blk, ngrp)
        assert blk_tot.max() <= cfg.GB, (
            f"core {k}: max groups/block {blk_tot.max()} > {cfg.GB}")
        # groups
        grp_node = np.repeat(nodes, ngrp)               # local node per group
        within = np.arange(len(grp_node), dtype=np.int64) - \
            np.repeat(cumg - ngrp, ngrp)                # 0..ngrp-1
        g_global = blk[grp_node] * cfg.GB + gbase_n[grp_node] + within
        nog[g_global] = grp_node
        glab[g_global] = (grp_node % 128).astype(np.float32)
        # edges -> slots
        n_e = ed                                        # local dst per edge
        j_in = np.arange(len(ed), dtype=np.int64) - estart[n_e]
        grel = gbase_n[n_e] + j_in // 8
        lab = grel % 16
        c = grel // 16
        p = lab + 16 * (j_in % 8)
        chunk = blk[n_e] * cfg.CPB + c
        src_slot[p, chunk] = es.astype(np.int32)
        # [p, T] layouts for the device
        nog_pt = nog.reshape(cfg.NT, 128).T.astype(np.int32).copy()
        glab_pt = glab.reshape(cfg.NT, 128).T.astype(np.float32).copy()
        # int16 dma_gather index layout for the per-piece d-gather:
        # call for piece p covers groups [p*GPP, (p+1)*GPP); sequence pos i
        # lives at [i%16, p*GPP//16 + i//16], replicated over 16-part groups
        gpp = cfg.PIECE_BLOCKS * cfg.GB          # groups per full piece
        nog16 = np.zeros((16, cfg.G // 16), np.int16)
        g_all = np.arange(cfg.G, dtype=np.int64)
        call = g_all // gpp
        i_in = g_all % gpp
        nog16[i_in % 16, call * (gpp // 16) + i_in // 16] = \
            nog.astype(np.int16)
        nog16_full = np.tile(nog16, (8, 1))
        out.append({
            "src_slot": src_slot,
            "nog": nog_pt,
            "nog16": nog16_full,
            "glabel": glab_pt,
        })
    return out


# ------------------------------------------------------------ bass build ---
def build_program(cfg):
    import concourse.bass as bass
    import concourse.bacc as bacc
    import concourse.mybir as mybir
    import concourse.tile as tile
    from concourse.bass import IndirectOffsetOnAxis

    dt = mybir.dt
    F32, BF, I32 = dt.float32, dt.bfloat16, dt.int32
    AF = mybir.ActivationFunctionType
    OP = mybir.AluOpType
    HID, ROW, RHSW, NCLS = cfg.HID, cfg.ROW, cfg.RHS_W, cfg.NCLS

    nc = bacc.Bacc("TRN2", target_bir_lowering=False, debug=False,
                   enable_asserts=False, num_devices=cfg.NCORES)

    # ---- I/O ----
    xT = nc.dram_tensor("xT", [cfg.N_IN, cfg.VPAD], F32, kind="ExternalInput")
    W1 = nc.dram_tensor("W1", [cfg.N_IN, HID], F32, kind="ExternalInput")
    W2 = nc.dram_tensor("W2", [HID, HID], F32, kind="ExternalInput")
    Wo = nc.dram_tensor("Wo", [HID, NCLS], F32, kind="ExternalInput")
    a1s = nc.dram_tensor("a1s", [HID], F32, kind="ExternalInput")
    a1d = nc.dram_tensor("a1d", [HID], F32, kind="ExternalInput")
    a2s = nc.dram_tensor("a2s", [HID], F32, kind="ExternalInput")
    a2d = nc.dram_tensor("a2d", [HID], F32, kind="ExternalInput")
    b1 = nc.dram_tensor("b1", [HID], F32, kind="ExternalInput")
    b2 = nc.dram_tensor("b2", [HID], F32, kind="ExternalInput")
    bo = nc.dram_tensor("bo", [NCLS], F32, kind="ExternalInput")
    src_slot = nc.dram_tensor("src_slot", [128, cfg.NCHUNK], I32,
                              kind="ExternalInput")
    nog_in = nc.dram_tensor("nog", [128, cfg.NT], I32, kind="ExternalInput")
    nog16_in = nc.dram_tensor("nog16", [128, cfg.G // 16], mybir.dt.int16,
                              kind="ExternalInput")
    glab_in = nc.dram_tensor("glabel", [128, cfg.NT], F32,
                             kind="ExternalInput")
    out_t = nc.dram_tensor("out", [cfg.VPAD, NCLS], F32,
                           kind="ExternalOutput")

    # ---- inline constants ----
    ident_d = nc.inline_tensor(np.eye(128, dtype=np.float32), "ident")
    mask_np = (np.arange(128)[:, None] % 16 == np.arange(16)[None, :])
    mask_d = nc.inline_tensor(mask_np.astype(BF16), "mask16")
    iota_d = nc.inline_tensor(
        np.tile(np.arange(128, dtype=np.float32), (128, 1)), "iota2d")
    sent_np = np.zeros((1, ROW), BF16)
    sent_np[0, HID + 1] = BF16(-10000.0)
    sent_d = nc.inline_tensor(sent_np, "sentrow")

    groups = [list(range(cfg.NCORES))]

    with tile.TileContext(nc, num_cores=cfg.NCORES) as tc:
        with (
            tc.tile_pool(name="const", bufs=1) as cp,
            tc.tile_pool(name="gath", bufs=2) as gp,
            tc.tile_pool(name="work", bufs=2) as wp,
            tc.tile_pool(name="gsum", bufs=6) as sp,
            tc.tile_pool(name="psum", bufs=2, space="PSUM") as pp,
            tc.tile_pool(name="dram", bufs=1, space="DRAM") as dp,
        ):
            # ======== constants to SBUF ========
            ident = cp.tile([128, 128], F32, tag="ident")
            nc.sync.dma_start(ident[:], ident_d[:, :])
            mask16 = cp.tile([128, 16], BF, tag="mask16")
            nc.sync.dma_start(mask16[:], mask_d[:, :])
            iota2 = cp.tile([128, 128], F32, tag="iota2")
            nc.sync.dma_start(iota2[:], iota_d[:, :])
            srcsb = cp.tile([128, cfg.NCHUNK], I32, tag="srcsb")
            nc.sync.dma_start(srcsb[:], src_slot[:, :])
            nogsb = cp.tile([128, cfg.NT], I32, tag="nogsb")
            nc.sync.dma_start(nogsb[:], nog_in[:, :])
            nog16sb = cp.tile([128, cfg.G // 16], mybir.dt.int16,
                              tag="nog16sb")
            nc.sync.dma_start(nog16sb[:], nog16_in[:, :])
            glabsb = cp.tile([128, cfg.NT], F32, tag="glabsb")
            nc.sync.dma_start(glabsb[:], glab_in[:, :])
            wosb = cp.tile([128, NCLS], F32, tag="wosb")
            nc.sync.dma_start(wosb[:], Wo[:, :])
            b1r = cp.tile([128, HID], F32, tag="b1r")
            nc.sync.dma_start(b1r[:], b1[None, :].to_broadcast([128, HID]))
            b2r = cp.tile([128, HID], F32, tag="b2r")
            nc.sync.dma_start(b2r[:], b2[None, :].to_broadcast([128, HID]))
            bor = cp.tile([128, NCLS], F32, tag="bor")
            nc.sync.dma_start(bor[:], bo[None, :].to_broadcast([128, NCLS]))

            def make_rhs(Wd, asd_s, asd_d, nchunks, tagbase):
                """rhs tiles [128, HID+2] = [W chunk | W@a_src | W@a_dst]."""
                asd = cp.tile([128, 2], F32, tag=tagbase + "_asd")
                nc.sync.dma_start(asd[:, 0:1], asd_s[:, None])
                nc.sync.dma_start(asd[:, 1:2], asd_d[:, None])
                tiles = []
                for c in range(nchunks):
                    rt = cp.tile([128, HID + 2], F32, tag=f"{tagbase}_{c}")
                    nc.sync.dma_start(rt[:, 0:HID],
                                      Wd[c * 128:(c + 1) * 128, :])
                    tp = pp.tile([128, 128], F32, tag="mm")
                    nc.tensor.transpose(tp[:], rt[:, 0:HID], ident[:])
                    wt = wp.tile([128, 128], F32, tag="wt")
                    nc.scalar.activation(wt[:], tp[:], AF.Copy)
                    sp2 = pp.tile([128, 2], F32, tag="mm")
                    nc.tensor.matmul(sp2[:], lhsT=wt[:], rhs=asd[:],
                                     start=True, stop=True)
                    nc.vector.tensor_copy(rt[:, HID:HID + 2], sp2[:])
                    tiles.append(rt)
                return tiles

            import os as _os
            _stage = int(_os.environ.get("GAT_STAGE", "6"))
            if _stage >= 1:
                rhs1 = make_rhs(W1, a1s, a1d, cfg.N_IN // 128, "rhs1")
                rhs2 = make_rhs(W2, a2s, a2d, 1, "rhs2")

            h2st = cp.tile([128, cfg.NB * HID], F32, tag="h2st")

            # persistent selector tiles: per chunk-pair layout is
            # [realA(16) | zeros(16) | realB(16)] so that the [*,32] lhsT
            # slice of either chunk has true zeros in its other half.
            # memset once; per-piece builds only touch the real columns.
            SELW = 48 * (cfg.PIECE_BLOCKS * cfg.CPB // 2)
            selA = cp.tile([128, SELW], BF, tag="selA")
            selB = cp.tile([128, SELW], BF, tag="selB")
            nc.gpsimd.memset(selA[:], 0.0)
            nc.gpsimd.memset(selB[:], 0.0)

            # DRAM scratch
            tables = [dp.tile([cfg.N + 1, ROW], BF, name=f"table{i}",
                              tag=f"table{i}") for i in range(2)]
            shards = [dp.tile([cfg.V, ROW], BF, name=f"shard{i}",
                              tag=f"shard{i}") for i in range(2)]
            d_drams = [dp.tile([cfg.VPAD, 1], F32, name=f"ddram{i}",
                               tag=f"ddram{i}") for i in range(2)]
            dgds = [dp.tile([cfg.G], F32, name=f"dgd{i}", tag=f"dgd{i}")
                    for i in range(2)]
            dsds = [dp.tile([cfg.SLOTS], F32, name=f"dsd{i}", tag=f"dsd{i}")
                    for i in range(2)]
            # d replicated to 64-wide rows so dma_gather (256B elems) can
            # fetch per-group d with int16 local node indices
            dreps = [dp.tile([cfg.VPAD, 64], F32, name=f"drep{i}",
                             tag=f"drep{i}") for i in range(2)]


            # ======== per-layer table prep ========
            def build_rows(ppre, b, shard, d_dram, drep):
                """ppre: psum [128, HID+2] = [h' | s | d] for block b."""
                rows = wp.tile([128, ROW], BF, tag="rows")
                nc.scalar.activation(rows[:, 0:HID], ppre[:, 0:HID], AF.Copy)
                nc.gpsimd.memset(rows[:, HID:HID + 1], 1.0)
                nc.scalar.activation(rows[:, HID + 1:HID + 2],
                                     ppre[:, HID:HID + 1], AF.Copy)
                shi = wp.tile([128, 1], F32, tag="shi")
                nc.vector.tensor_copy(shi[:], rows[:, HID + 1:HID + 2])
                nc.vector.tensor_tensor(rows[:, HID + 2:HID + 3],
                                        ppre[:, HID:HID + 1], shi[:],
                                        op=OP.subtract)
                dcol = wp.tile([128, 1], F32, tag="dcol")
                nc.vector.tensor_copy(dcol[:], ppre[:, HID + 1:HID + 2])
                nc.sync.dma_start(d_dram[b * 128:(b + 1) * 128, :], dcol[:])
                d64 = wp.tile([128, 64], F32, tag="d64")
                nc.vector.tensor_copy(d64[:], dcol[:].to_broadcast([128, 64]))
                nc.sync.dma_start(drep[b * 128:(b + 1) * 128, :], d64[:])
                nrow = min(128, cfg.V - b * 128)
                nc.sync.dma_start(shard[b * 128:b * 128 + nrow, :],
                                  rows[0:nrow, :])

            def prep_layer1():
                for b in range(cfg.NB):
                    ppre = pp.tile([128, HID + 2], F32, tag="mm")
                    for c in range(cfg.N_IN // 128):
                        xt = wp.tile([128, 128], F32, tag="xt")
                        nc.sync.dma_start(
                            xt[:],
                            xT[c * 128:(c + 1) * 128,
                               b * 128:(b + 1) * 128])
                        nc.tensor.matmul(ppre[:], lhsT=xt[:], rhs=rhs1[c][:],
                                         start=(c == 0),
                                         stop=(c == cfg.N_IN // 128 - 1))
                    build_rows(ppre, b, shards[0], d_drams[0], dreps[0])

            def prep_layer2():
                for b in range(cfg.NB):
                    tp = pp.tile([128, 128], F32, tag="mm")
                    nc.tensor.transpose(
                        tp[:], h2st[:, b * HID:(b + 1) * HID], ident[:])
                    h2T = wp.tile([128, 128], F32, tag="h2T")
                    nc.scalar.activation(h2T[:], tp[:], AF.Copy)
                    ppre = pp.tile([128, HID + 2], F32, tag="mm")
                    nc.tensor.matmul(ppre[:], lhsT=h2T[:], rhs=rhs2[0][:],
                                     start=True, stop=True)
                    build_rows(ppre, b, shards[1], d_drams[1], dreps[1])

            def allgather(li):
                nc.gpsimd.collective_compute(
                    "AllGather", mybir.AluOpType.bypass,
                    replica_groups=groups,
                    ins=[shards[li][:, :].opt()],
                    outs=[tables[li][0:cfg.N, :].opt()],
                )
                nc.sync.dma_start(tables[li][cfg.N:cfg.N + 1, :],
                                  sent_d[:, :])

            # ======== main per-layer loop ========
            def main_layer(li, epilogue):
                table, d_dram = tables[li], d_drams[li]
                dgd, dsd = dgds[li], dsds[li]
                drep = dreps[li]
                PB = cfg.PIECE_BLOCKS
                gpp16 = (PB * cfg.GB) // 16
                for pi, (b0, nb) in enumerate(cfg.pieces):
                    Tn = nb * cfg.TPB
                    NCh = nb * cfg.CPB
                    Gn = nb * cfg.GB
                    gbase = b0 * cfg.GB
                    # d per group: one bulk int16 dma_gather per piece
                    dg = wp.tile([128, PB * cfg.TPB * 64], F32, tag="dg")
                    dg3 = dg[:, 0:Tn * 64].rearrange(
                        "p (t e) -> p t e", e=64)
                    nidx = wp.tile([128, gpp16], mybir.dt.int16, tag="nidx")
                    nc.vector.tensor_copy(
                        nidx[:, 0:Gn // 16],
                        nog16sb[:, pi * gpp16:pi * gpp16 + Gn // 16])
                    nc.gpsimd.dma_gather(
                        out_ap=dg3,
                        in_ap=drep[:, :],
                        idxs_ap=nidx[:, 0:Gn // 16],
                        num_idxs=Gn,
                        num_idxs_reg=Gn,
                        elem_size=64,
                        single_packet=False)
                    nc.sync.dma_start(
                        dgd[gbase:gbase + Gn].rearrange("(t p) -> p t", p=128),
                        dg3[:, :, 0:1].rearrange("p t e -> p (t e)"))
                    nc.sync.dma_start(
                        dsd[b0 * cfg.CPB * 128:(b0 * cfg.CPB + NCh) * 128]
                        .rearrange("(bb c i l) -> bb c i l",
                                   bb=nb, c=cfg.CPB, i=8, l=16),
                        dgd[gbase:gbase + Gn]
                        .rearrange("(bb c l) -> bb c () l",
                                   bb=nb, c=cfg.CPB, l=16)
                        .to_broadcast([nb, cfg.CPB, 8, 16]))
                    dsl = wp.tile([128, PB * cfg.CPB], F32, tag="dsl")
                    nc.sync.dma_start(
                        dsl[:, 0:NCh],
                        dsd[b0 * cfg.CPB * 128:(b0 * cfg.CPB + NCh) * 128]
                        .rearrange("(j p) -> p j", p=128))
                    # feature gather: one 128-row indirect DMA per slot-chunk
                    gt = gp.tile([128, ROW * PB * cfg.CPB], BF, tag="gt")
                    for j in range(NCh):
                        nc.gpsimd.indirect_dma_start(
                            out=gt[:, ROW * j:ROW * (j + 1)], out_offset=None,
                            in_=table[:, :],
                            in_offset=IndirectOffsetOnAxis(
                                ap=srcsb[:, b0 * cfg.CPB + j:
                                         b0 * cfg.CPB + j + 1],
                                axis=0))
                    gv = gt[:, 0:ROW * NCh].rearrange(
                        "p (j c) -> p j c", c=ROW)
                    # phase A: ex = exp(leaky_relu(s_src + d_dst))
                    sf = wp.tile([128, PB * cfg.CPB], F32, tag="sf")
                    sfv = sf[:, 0:NCh].rearrange("p j -> p j ()")
                    nc.vector.tensor_tensor(
                        sfv, gv[:, :, HID + 1:HID + 2],
                        gv[:, :, HID + 2:HID + 3], op=OP.add)
                    ep = wp.tile([128, PB * cfg.CPB], F32, tag="ep")
                    nc.vector.tensor_tensor(ep[:, 0:NCh], sf[:, 0:NCh],
                                            dsl[:, 0:NCh], op=OP.add)
                    es = wp.tile([128, PB * cfg.CPB], F32, tag="es")
                    nc.vector.tensor_scalar_mul(es[:, 0:NCh], ep[:, 0:NCh],
                                                0.2)
                    el = wp.tile([128, PB * cfg.CPB], F32, tag="el")
                    nc.vector.tensor_tensor(el[:, 0:NCh], ep[:, 0:NCh],
                                            es[:, 0:NCh], op=OP.max)
                    exf = wp.tile([128, PB * cfg.CPB], F32, tag="exf")
                    nc.scalar.activation(exf[:, 0:NCh], el[:, 0:NCh], AF.Exp)
                    exb = wp.tile([128, PB * cfg.CPB], BF, tag="exb")
                    nc.vector.tensor_copy(exb[:, 0:NCh], exf[:, 0:NCh])
                    # selector build: even chunks -> cols [48a, 48a+16),
                    # odd chunks -> cols [48a+32, 48a+48)
                    sel1 = selA if (b0 // cfg.PIECE_BLOCKS) % 2 == 0 else selB
                    npair = NCh // 2
                    exv = exb[:, 0:NCh].rearrange("p (a u) -> p a u", u=2)
                    maskv = mask16[:].rearrange("p l -> p () l") \
                        .to_broadcast([128, npair, 16])
                    selv = sel1[:, 0:48 * npair].rearrange(
                        "p (a w) -> p a w", w=48)
                    nc.vector.tensor_tensor(
                        selv[:, :, 0:16],
                        exv[:, :, 0:1].to_broadcast([128, npair, 16]),
                        maskv, op=OP.mult)
                    nc.vector.tensor_tensor(
                        selv[:, :, 32:48],
                        exv[:, :, 1:2].to_broadcast([128, npair, 16]),
                        maskv, op=OP.mult)
                    # level 1 + level 2
                    for bb in range(nb):
                        b = b0 + bb
                        pl2 = pp.tile([128, RHSW], F32, tag="l2")
                        for t in range(cfg.TPB):
                            pl1 = pp.tile([128, RHSW], F32, tag="l1")
                            for al in range(4):
                                for u in range(2):
                                    j = bb * cfg.CPB + t * 8 + 2 * al + u
                                    A = j // 2
                                    nc.tensor.matmul(
                                        pl1[32 * al:32 * al + 32, :],
                                        lhsT=sel1[:, 48 * A + 16 * u:
                                                  48 * A + 16 * u + 32],
                                        rhs=gt[:, ROW * j:ROW * j + RHSW],
                                        start=(u == 0), stop=(u == 1),
                                        tile_position=(0, 32 * al))
                            gs = sp.tile([128, RHSW], F32, tag="gsum")
                            nc.scalar.activation(gs[:], pl1[:], AF.Copy)
                            T = b * cfg.TPB + t
                            l2s = wp.tile([128, 128], F32, tag="l2s")
                            nc.vector.tensor_tensor(
                                l2s[:],
                                glabsb[:, T:T + 1].to_broadcast([128, 128]),
                                iota2[:], op=OP.is_equal)
                            nc.tensor.matmul(pl2[:], lhsT=l2s[:], rhs=gs[:],
                                             start=(t == 0),
                                             stop=(t == cfg.TPB - 1))
                        epilogue(b, pl2)

            def epi_norm(pl2, brow):
                den = wp.tile([128, 1], F32, tag="den")
                nc.vector.tensor_scalar_max(den[:], pl2[:, HID:HID + 1],
                                            1e-30)
                rec = wp.tile([128, 1], F32, tag="rec")
                nc.vector.reciprocal(rec[:], den[:])
                hb = wp.tile([128, HID], F32, tag="hb")
                nc.vector.tensor_scalar_mul(hb[:], pl2[:, 0:HID],
                                            rec[:, 0:1])
                nc.vector.tensor_tensor(hb[:], hb[:], brow[:], op=OP.add)
                return hb

            def epilogue1(b, pl2):
                hb = epi_norm(pl2, b1r)
                nc.scalar.activation(h2st[:, b * HID:(b + 1) * HID], hb[:],
                                     AF.Relu)

            def epilogue2(b, pl2):
                hb = epi_norm(pl2, b2r)
                h3 = wp.tile([128, HID], F32, tag="h3")
                nc.scalar.activation(h3[:], hb[:], AF.Relu)
                tp = pp.tile([128, 128], F32, tag="mm")
                nc.tensor.transpose(tp[:], h3[:], ident[:])
                h3T = wp.tile([128, 128], F32, tag="h3T")
                nc.scalar.activation(h3T[:], tp[:], AF.Copy)
                po = pp.tile([128, NCLS], F32, tag="mm")
                nc.tensor.matmul(po[:], lhsT=h3T[:], rhs=wosb[:],
                                 start=True, stop=True)
                lg = wp.tile([128, NCLS], F32, tag="lg")
                nc.vector.tensor_tensor(lg[:], po[:], bor[:], op=OP.add)
                mx = wp.tile([128, 1], F32, tag="mx")
                nc.vector.tensor_reduce(mx[:], lg[:],
                                        axis=mybir.AxisListType.X, op=OP.max)
                lgs = wp.tile([128, NCLS], F32, tag="lgs")
                nc.vector.tensor_scalar(lgs[:], lg[:], mx[:, 0:1], None,
                                        op0=OP.subtract)
                pe = wp.tile([128, NCLS], F32, tag="pe")
                rs = wp.tile([128, 1], F32, tag="rs")
                nc.scalar.activation(pe[:], lgs[:], AF.Exp,
                                     accum_out=rs[:, 0:1])
                rr = wp.tile([128, 1], F32, tag="rr")
                nc.vector.reciprocal(rr[:], rs[:])
                ot = wp.tile([128, NCLS], F32, tag="ot")
                nc.vector.tensor_scalar_mul(ot[:], pe[:], rr[:, 0:1])
                nc.sync.dma_start(out_t[b * 128:(b + 1) * 128, :], ot[:])

            # ======== schedule ========
            stage = _stage
            if stage >= 2:
                prep_layer1()
            if stage >= 3:
                allgather(0)
            if stage >= 4:
                main_layer(0, epilogue1)
            if stage >= 5:
                prep_layer2()
                allgather(1)
            if stage >= 6:
                main_layer(1, epilogue2)
            if stage < 6:
                # touch the output so the program remains well formed
                zz = wp.tile([128, NCLS], F32, tag="zz")
                nc.gpsimd.memset(zz[:], 0.0)
                for b in range(cfg.NB):
                    nc.sync.dma_start(out_t[b * 128:(b + 1) * 128, :], zz[:])

    nc._gat_dbg = {
        "table0": tables[0].tensor.name, "table1": tables[1].tensor.name,
        "shard0": shards[0].tensor.name, "shard1": shards[1].tensor.name,
        "ddram0": d_drams[0].tensor.name, "ddram1": d_drams[1].tensor.name,
        "dgd0": dgds[0].tensor.name, "dsd0": dsds[0].tensor.name,
    }
    nc.compile()
    return nc


# ------------------------------------------------------------------ run ---
_PROG_CACHE = {}


def _get_program(cfg):
    key = (cfg.N, cfg.E, cfg.NCORES)
    if key not in _PROG_CACHE:
        _PROG_CACHE[key] = build_program(cfg)
    return _PROG_CACHE[key]


_PRE_CACHE = {}


def run(cfg, inputs, trace=False):
    from concourse.bass_utils import run_bass_kernel_spmd

    nc = _get_program(cfg)
    ei = np.asarray(inputs["edge_index"])
    pkey = (cfg.N, cfg.E, int(ei[0, :16].sum()), int(ei[1, -16:].sum()))
    if pkey not in _PRE_CACHE:
        _PRE_CACHE[pkey] = preprocess(cfg, ei)
    pre = _PRE_CACHE[pkey]
    in_maps = _make_in_maps(cfg, inputs, pre)
    res = run_bass_kernel_spmd(nc, in_maps, core_ids=list(range(cfg.NCORES)),
                               trace=trace)
    outs = [res.results[k]["out"][:cfg.V] for k in range(cfg.NCORES)]
    full = np.concatenate(outs, axis=0).astype(np.float32)
    return full, res


def timed_run(cfg, inputs, iters=3):
    """Time device execution with device-resident inputs (axon transfer
    excluded). Returns (best_seconds, outputs_of_last_iter)."""
    import time

    import jax
    from jax.sharding import Mesh, PartitionSpec
    from jax.experimental.shard_map import shard_map
    import concourse.mybir as mybir
    from concourse import bass2jax
    from concourse.bass2jax import _bass_exec_p, partition_id_tensor

    bass2jax.install_neuronx_cc_hook()
    nc = _get_program(cfg)
    ei = np.asarray(inputs["edge_index"])
    pkey = (cfg.N, cfg.E, int(ei[0, :16].sum()), int(ei[1, -16:].sum()))
    if pkey not in _PRE_CACHE:
        _PRE_CACHE[pkey] = preprocess(cfg, ei)
    pre = _PRE_CACHE[pkey]
    in_maps = _make_in_maps(cfg, inputs, pre)

    partition_name = (nc.partition_id_tensor.name
                      if nc.partition_id_tensor else None)
    in_names, out_names, out_avals, zero_outs = [], [], [], []
    for alloc in nc.m.functions[0].allocations:
        if not isinstance(alloc, mybir.MemoryLocationSet):
            continue
        name = alloc.memorylocations[0].name
        if alloc.kind == "ExternalInput":
            if name != partition_name:
                in_names.append(name)
        elif alloc.kind == "ExternalOutput":
            out_names.append(name)
            shape = tuple(alloc.tensor_shape)
            dtype = mybir.dt.np(alloc.dtype)
            out_avals.append(jax.core.ShapedArray(shape, dtype))
            zero_outs.append(np.zeros(shape, dtype))
    n_params = len(in_names)
    n_outs = len(out_avals)
    all_in_names = list(in_names) + list(out_names)
    if partition_name is not None:
        all_in_names.append(partition_name)
    donate = tuple(range(n_params, n_params + n_outs))

    def _body(*args):
        operands = list(args)
        if partition_name is not None:
            operands.append(partition_id_tensor())
        outs = _bass_exec_p.bind(
            *operands, out_avals=tuple(out_avals),
            in_names=tuple(all_in_names), out_names=tuple(out_names),
            lowering_input_output_aliases=(),
            sim_require_finite=True, sim_require_nnan=True, nc=nc)
        return tuple(outs)

    devices = jax.devices()[:cfg.NCORES]
    mesh = Mesh(np.asarray(devices), ("core",))
    in_specs = (PartitionSpec("core"),) * (n_params + n_outs)
    out_specs = (PartitionSpec("core"),) * len(out_names)
    sharded = jax.jit(
        shard_map(_body, mesh=mesh, in_specs=in_specs, out_specs=out_specs,
                  check_rep=False),
        donate_argnums=donate, keep_unused=True)
    concat_in = [
        np.concatenate([np.asarray(in_maps[c][nm]) for c in range(cfg.NCORES)],
                       axis=0)
        for nm in in_names]
    sharding = jax.sharding.NamedSharding(mesh, PartitionSpec("core"))
    dev_in = [jax.device_put(a, sharding) for a in concat_in]
    times = []
    out_arrs = None
    for _ in range(iters):
        zo = [jax.device_put(
            np.zeros((cfg.NCORES * z.shape[0], *z.shape[1:]), z.dtype),
            sharding) for z in zero_outs]
        jax.block_until_ready(zo)
        t0 = time.time()
        out_arrs = sharded(*dev_in, *zo)
        jax.block_until_ready(out_arrs)
        times.append(time.time() - t0)
    oi = out_names.index("out")
    full = np.asarray(out_arrs[oi]).reshape(cfg.NCORES, cfg.VPAD, cfg.NCLS)
    out = np.concatenate([full[c, :cfg.V] for c in range(cfg.NCORES)], axis=0)
    return min(times), out.astype(np.float32)


def _make_in_maps(cfg, inputs, pre):
    x = np.asarray(inputs["x"], np.float32)
    common = {
        "W1": np.asarray(inputs["W1"], np.float32),
        "W2": np.asarray(inputs["W2"], np.float32),
        "Wo": np.asarray(inputs["Wo"], np.float32),
        "a1s": np.asarray(inputs["a1_src"], np.float32),
        "a1d": np.asarray(inputs["a1_dst"], np.float32),
        "a2s": np.asarray(inputs["a2_src"], np.float32),
        "a2d": np.asarray(inputs["a2_dst"], np.float32),
        "b1": np.asarray(inputs["b1"], np.float32),
        "b2": np.asarray(inputs["b2"], np.float32),
        "bo": np.asarray(inputs["bo"], np.float32),
    }
    in_maps = []
    for k in range(cfg.NCORES):
        xs = x[k * cfg.V:(k + 1) * cfg.V]
        xT = np.zeros((cfg.N_IN, cfg.VPAD), np.float32)
        xT[:, :cfg.V] = xs.T
        m = dict(common)
        m["xT"] = np.ascontiguousarray(xT)
        m["src_slot"] = pre[k]["src_slot"]
        m["nog"] = pre[k]["nog"]
        m["nog16"] = pre[k]["nog16"]
        m["glabel"] = pre[k]["glabel"]
        in_maps.append(m)
    return in_maps


def kernel(**inputs):
    cfg = DEFAULT_CFG
    full, _ = run(cfg, inputs, trace=False)
    return full


# revision 35
# speedup vs baseline: 1.2497x; 1.0527x over previous
"""Trainium2 Bass kernel for a 2-layer GAT (heads=1) + linear head + softmax.

Strategy (8 NeuronCores, graph/data parallel):
  - Nodes sharded across cores (12500 dst nodes each); edges partitioned by
    destination node so segment softmax / scatter stay local to a core.
  - Per layer, each core computes projected features for its node shard:
    table row = [h' (HID, bf16) | 1.0 | s_hi | s_lo]  (s = h' @ a_src split
    into two bf16 halves for ~f32 precision), then an AllGather replicates
    the full node table to every core's DRAM (halo exchange).
  - Edges are laid out in "slots": 8 slots per group, 16 groups per
    128-slot chunk, 24 chunks per 128-node destination block (384 groups =
    3 "gsum tiles" per block, padded -> identical program on every core).
  - Main loop per layer: one big indirect-DMA gather of [h'|1|s] rows per
    piece, per-edge attention logits e = leaky_relu(s_src + d_dst) and
    ex = exp(e) (no max-shift needed; logits are bounded), then a two-level
    matmul segment-reduction:
      level 1: ex-carrying selector (static 16-label mask) x gathered rows
               -> per-group partial [sum(ex*h) | sum(ex)]
      level 2: is_equal(group-label, node-iota) selector x group partials
               -> per-node [numerator | denominator] accumulated in PSUM.
    Epilogue divides by the denominator (softmax normalization), adds bias,
    applies relu; layer 2 additionally applies the output head + softmax.
"""

import math
import sys

import numpy as np

if "/opt/trn_rl_repo" not in sys.path:
    sys.path.insert(0, "/opt/trn_rl_repo")

import ml_dtypes

BF16 = ml_dtypes.bfloat16


# ---------------------------------------------------------------- config ---
class Cfg:
    def __init__(self, N, E, n_in=256, hid=128, ncls=3, ncores=8,
                 piece_blocks=5, gb=384):
        self.N, self.E = N, E
        self.N_IN, self.HID, self.NCLS = n_in, hid, ncls
        self.NCORES = ncores
        assert N % ncores == 0
        self.V = N // ncores                      # real dst nodes per core
        self.NB = math.ceil(self.V / 128)         # node blocks per core
        self.VPAD = self.NB * 128
        self.GB = gb                              # groups per block (padded)
        assert gb % 16 == 0
        self.CPB = gb // 16                       # chunks per block
        self.G = self.NB * self.GB                # groups per core
        assert self.G % 128 == 0
        self.NT = self.G // 128                   # gsum tiles per core (NB*3)
        self.TPB = self.GB // 128                 # gsum tiles per block
        assert self.GB % 128 == 0
        self.NCHUNK = self.NB * self.CPB
        self.SLOTS = self.NCHUNK * 128
        self.SENT = N                             # sentinel table row index
        self.ROW = hid + 3                        # h | one | s_hi | s_lo
        self.RHS_W = hid + 1                      # matmul rhs width (h | one)
        # pieces: (block_start, nblocks)
        self.pieces = []
        b = 0
        while b < self.NB:
            nb = min(piece_blocks, self.NB - b)
            self.pieces.append((b, nb))
            b += nb
        self.PIECE_BLOCKS = piece_blocks


DEFAULT_CFG = Cfg(N=100000, E=1600000)


# ---------------------------------------------------- host preprocessing ---
def preprocess(cfg, edge_index):
    """Partition edges by destination core and build per-core slot layout.

    Returns per-core dict of int/float index tensors (identical shapes on
    every core so one NEFF serves all 8).
    """
    src = np.concatenate([edge_index[0], np.arange(cfg.N, dtype=np.int32)])
    dst = np.concatenate([edge_index[1], np.arange(cfg.N, dtype=np.int32)])
    order = np.argsort(dst, kind="stable")
    src, dst = src[order].astype(np.int64), dst[order].astype(np.int64)
    core_of = dst // cfg.V
    bounds = np.searchsorted(core_of, np.arange(cfg.NCORES + 1))
    out = []
    for k in range(cfg.NCORES):
        lo, hi = bounds[k], bounds[k + 1]
        es = src[lo:hi]
        ed = dst[lo:hi] - k * cfg.V               # local dst, sorted
        deg = np.bincount(ed, minlength=cfg.V).astype(np.int64)
        estart = np.zeros(cfg.V + 1, np.int64)
        np.cumsum(deg, out=estart[1:])
        ngrp = (deg + 7) // 8                     # >=1 (self loops)

        src_slot = np.full((128, cfg.NCHUNK), cfg.SENT, np.int32)
        nog = np.full(cfg.G, cfg.VPAD - 1, np.int64)   # node of group
        glab = np.full(cfg.G, 1e9, np.float32)         # in-block node label

        # vectorized slot/group layout
        nodes = np.arange(cfg.V, dtype=np.int64)
        blk = nodes // 128
        # group base of each node within its block
        cumg = np.cumsum(ngrp)
        blk_start_node = blk * 128
        cumg_before_block = np.where(blk_start_node > 0,
                                     cumg[blk_start_node - 1], 0)
        gbase_n = (cumg - ngrp) - cumg_before_block
        blk_tot = np.zeros(cfg.NB, np.int64)
        np.add.at(blk_tot, blk, ngrp)
        assert blk_tot.max() <= cfg.GB, (
            f"core {k}: max groups/block {blk_tot.max()} > {cfg.GB}")
        # groups
        grp_node = np.repeat(nodes, ngrp)               # local node per group
        within = np.arange(len(grp_node), dtype=np.int64) - \
            np.repeat(cumg - ngrp, ngrp)                # 0..ngrp-1
        g_global = blk[grp_node] * cfg.GB + gbase_n[grp_node] + within
        nog[g_global] = grp_node
        glab[g_global] = (grp_node % 128).astype(np.float32)
        # edges -> slots
        n_e = ed                                        # local dst per edge
        j_in = np.arange(len(ed), dtype=np.int64) - estart[n_e]
        grel = gbase_n[n_e] + j_in // 8
        lab = grel % 16
        c = grel // 16
        p = lab + 16 * (j_in % 8)
        chunk = blk[n_e] * cfg.CPB + c
        src_slot[p, chunk] = es.astype(np.int32)
        # [p, T] layouts for the device
        nog_pt = nog.reshape(cfg.NT, 128).T.astype(np.int32).copy()
        glab_pt = glab.reshape(cfg.NT, 128).T.astype(np.float32).copy()
        # int16 dma_gather index layout for the per-piece d-gather:
        # call for piece p covers groups [p*GPP, (p+1)*GPP); sequence pos i
        # lives at [i%16, p*GPP//16 + i//16], replicated over 16-part groups
        gpp = cfg.PIECE_BLOCKS * cfg.GB          # groups per full piece
        nog16 = np.zeros((16, cfg.G // 16), np.int16)
        g_all = np.arange(cfg.G, dtype=np.int64)
        call = g_all // gpp
        i_in = g_all % gpp
        nog16[i_in % 16, call * (gpp // 16) + i_in // 16] = \
            nog.astype(np.int16)
        nog16_full = np.tile(nog16, (8, 1))
        out.append({
            "src_slot": src_slot,
            "nog": nog_pt,
            "nog16": nog16_full,
            "glabel": glab_pt,
            "blk_tot": blk_tot.copy(),
        })
    return out


def _used_chunks(cfg, pre):
    """Per-block chunk count actually carrying edges, maxed over cores (the
    NEFF is shared), rounded up to whole 16-group chunks."""
    mx = np.maximum.reduce([p["blk_tot"] for p in pre])
    return tuple(int(x) for x in np.minimum((mx + 15) // 16, cfg.CPB))


# ------------------------------------------------------------ bass build ---
def build_program(cfg, used_chunks=None):
    import concourse.bass as bass
    import concourse.bacc as bacc
    import concourse.mybir as mybir
    import concourse.tile as tile
    from concourse.bass import IndirectOffsetOnAxis

    dt = mybir.dt
    F32, BF, I32 = dt.float32, dt.bfloat16, dt.int32
    AF = mybir.ActivationFunctionType
    OP = mybir.AluOpType
    HID, ROW, RHSW, NCLS = cfg.HID, cfg.ROW, cfg.RHS_W, cfg.NCLS

    if used_chunks is None:
        used_chunks = (cfg.CPB,) * cfg.NB
    nc = bacc.Bacc("TRN2", target_bir_lowering=False, debug=False,
                   enable_asserts=False, num_devices=cfg.NCORES)

    # ---- I/O ----
    xT = nc.dram_tensor("xT", [cfg.N_IN, cfg.VPAD], F32, kind="ExternalInput")
    W1 = nc.dram_tensor("W1", [cfg.N_IN, HID], F32, kind="ExternalInput")
    W2 = nc.dram_tensor("W2", [HID, HID], F32, kind="ExternalInput")
    Wo = nc.dram_tensor("Wo", [HID, NCLS], F32, kind="ExternalInput")
    a1s = nc.dram_tensor("a1s", [HID], F32, kind="ExternalInput")
    a1d = nc.dram_tensor("a1d", [HID], F32, kind="ExternalInput")
    a2s = nc.dram_tensor("a2s", [HID], F32, kind="ExternalInput")
    a2d = nc.dram_tensor("a2d", [HID], F32, kind="ExternalInput")
    b1 = nc.dram_tensor("b1", [HID], F32, kind="ExternalInput")
    b2 = nc.dram_tensor("b2", [HID], F32, kind="ExternalInput")
    bo = nc.dram_tensor("bo", [NCLS], F32, kind="ExternalInput")
    src_slot = nc.dram_tensor("src_slot", [128, cfg.NCHUNK], I32,
                              kind="ExternalInput")
    nog_in = nc.dram_tensor("nog", [128, cfg.NT], I32, kind="ExternalInput")
    nog16_in = nc.dram_tensor("nog16", [128, cfg.G // 16], mybir.dt.int16,
                              kind="ExternalInput")
    glab_in = nc.dram_tensor("glabel", [128, cfg.NT], F32,
                             kind="ExternalInput")
    out_t = nc.dram_tensor("out", [cfg.VPAD, NCLS], F32,
                           kind="ExternalOutput")

    # ---- inline constants ----
    ident_d = nc.inline_tensor(np.eye(128, dtype=np.float32), "ident")
    mask_np = (np.arange(128)[:, None] % 16 == np.arange(16)[None, :])
    mask_d = nc.inline_tensor(mask_np.astype(BF16), "mask16")
    iota_d = nc.inline_tensor(
        np.tile(np.arange(128, dtype=np.float32), (128, 1)), "iota2d")
    sent_np = np.zeros((1, ROW), BF16)
    sent_np[0, HID + 1] = BF16(-10000.0)
    sent_d = nc.inline_tensor(sent_np, "sentrow")

    groups = [list(range(cfg.NCORES))]

    with tile.TileContext(nc, num_cores=cfg.NCORES) as tc:
        with (
            tc.tile_pool(name="const", bufs=1) as cp,
            tc.tile_pool(name="gath", bufs=2) as gp,
            tc.tile_pool(name="work", bufs=2) as wp,
            tc.tile_pool(name="gsum", bufs=6) as sp,
            tc.tile_pool(name="psum", bufs=2, space="PSUM") as pp,
            tc.tile_pool(name="dram", bufs=1, space="DRAM") as dp,
        ):
            # ======== constants to SBUF ========
            ident = cp.tile([128, 128], F32, tag="ident")
            nc.sync.dma_start(ident[:], ident_d[:, :])
            mask16 = cp.tile([128, 16], BF, tag="mask16")
            nc.sync.dma_start(mask16[:], mask_d[:, :])
            iota2 = cp.tile([128, 128], F32, tag="iota2")
            nc.sync.dma_start(iota2[:], iota_d[:, :])
            srcsb = cp.tile([128, cfg.NCHUNK], I32, tag="srcsb")
            nc.sync.dma_start(srcsb[:], src_slot[:, :])
            nogsb = cp.tile([128, cfg.NT], I32, tag="nogsb")
            nc.sync.dma_start(nogsb[:], nog_in[:, :])
            nog16sb = cp.tile([128, cfg.G // 16], mybir.dt.int16,
                              tag="nog16sb")
            nc.sync.dma_start(nog16sb[:], nog16_in[:, :])
            glabsb = cp.tile([128, cfg.NT], F32, tag="glabsb")
            nc.sync.dma_start(glabsb[:], glab_in[:, :])
            wosb = cp.tile([128, NCLS], F32, tag="wosb")
            nc.sync.dma_start(wosb[:], Wo[:, :])
            b1r = cp.tile([128, HID], F32, tag="b1r")
            nc.sync.dma_start(b1r[:], b1[None, :].to_broadcast([128, HID]))
            b2r = cp.tile([128, HID], F32, tag="b2r")
            nc.sync.dma_start(b2r[:], b2[None, :].to_broadcast([128, HID]))
            bor = cp.tile([128, NCLS], F32, tag="bor")
            nc.sync.dma_start(bor[:], bo[None, :].to_broadcast([128, NCLS]))

            def make_rhs(Wd, asd_s, asd_d, nchunks, tagbase):
                """rhs tiles [128, HID+2] = [W chunk | W@a_src | W@a_dst]."""
                asd = cp.tile([128, 2], F32, tag=tagbase + "_asd")
                nc.sync.dma_start(asd[:, 0:1], asd_s[:, None])
                nc.sync.dma_start(asd[:, 1:2], asd_d[:, None])
                tiles = []
                for c in range(nchunks):
                    rt = cp.tile([128, HID + 2], F32, tag=f"{tagbase}_{c}")
                    nc.sync.dma_start(rt[:, 0:HID],
                                      Wd[c * 128:(c + 1) * 128, :])
                    tp = pp.tile([128, 128], F32, tag="mm")
                    nc.tensor.transpose(tp[:], rt[:, 0:HID], ident[:])
                    wt = wp.tile([128, 128], F32, tag="wt")
                    nc.scalar.activation(wt[:], tp[:], AF.Copy)
                    sp2 = pp.tile([128, 2], F32, tag="mm")
                    nc.tensor.matmul(sp2[:], lhsT=wt[:], rhs=asd[:],
                                     start=True, stop=True)
                    nc.vector.tensor_copy(rt[:, HID:HID + 2], sp2[:])
                    tiles.append(rt)
                return tiles

            import os as _os
            _stage = int(_os.environ.get("GAT_STAGE", "6"))
            if _stage >= 1:
                rhs1 = make_rhs(W1, a1s, a1d, cfg.N_IN // 128, "rhs1")
                rhs2 = make_rhs(W2, a2s, a2d, 1, "rhs2")

            h2st = cp.tile([128, cfg.NB * HID], F32, tag="h2st")

            # persistent selector tiles: per chunk-pair layout is
            # [realA(16) | zeros(16) | realB(16)] so that the [*,32] lhsT
            # slice of either chunk has true zeros in its other half.
            # memset once; per-piece builds only touch the real columns.
            SELW = 48 * (cfg.PIECE_BLOCKS * cfg.CPB // 2)
            selA = cp.tile([128, SELW], BF, tag="selA")
            selB = cp.tile([128, SELW], BF, tag="selB")
            nc.gpsimd.memset(selA[:], 0.0)
            nc.gpsimd.memset(selB[:], 0.0)
            GTW = cfg.ROW * cfg.PIECE_BLOCKS * cfg.CPB
            gtA = cp.tile([128, GTW], BF, tag="gtA")
            gtB = cp.tile([128, GTW], BF, tag="gtB")
            nc.gpsimd.memset(gtA[:], 0.0)
            nc.gpsimd.memset(gtB[:], 0.0)

            # DRAM scratch
            tables = [dp.tile([cfg.N + 1, ROW], BF, name=f"table{i}",
                              tag=f"table{i}") for i in range(2)]
            shards = [dp.tile([cfg.V, ROW], BF, name=f"shard{i}",
                              tag=f"shard{i}") for i in range(2)]
            d_drams = [dp.tile([cfg.VPAD, 1], F32, name=f"ddram{i}",
                               tag=f"ddram{i}") for i in range(2)]
            dgds = [dp.tile([cfg.G], F32, name=f"dgd{i}", tag=f"dgd{i}")
                    for i in range(2)]
            dsds = [dp.tile([cfg.SLOTS], F32, name=f"dsd{i}", tag=f"dsd{i}")
                    for i in range(2)]
            # d replicated to 64-wide rows so dma_gather (256B elems) can
            # fetch per-group d with int16 local node indices
            dreps = [dp.tile([cfg.VPAD, 64], F32, name=f"drep{i}",
                             tag=f"drep{i}") for i in range(2)]


            # ======== per-layer table prep ========
            def build_rows(ppre, b, shard, d_dram, drep):
                """ppre: psum [128, HID+2] = [h' | s | d] for block b."""
                rows = wp.tile([128, ROW], BF, tag="rows")
                nc.scalar.activation(rows[:, 0:HID], ppre[:, 0:HID], AF.Copy)
                nc.gpsimd.memset(rows[:, HID:HID + 1], 1.0)
                nc.scalar.activation(rows[:, HID + 1:HID + 2],
                                     ppre[:, HID:HID + 1], AF.Copy)
                shi = wp.tile([128, 1], F32, tag="shi")
                nc.vector.tensor_copy(shi[:], rows[:, HID + 1:HID + 2])
                nc.vector.tensor_tensor(rows[:, HID + 2:HID + 3],
                                        ppre[:, HID:HID + 1], shi[:],
                                        op=OP.subtract)
                dcol = wp.tile([128, 1], F32, tag="dcol")
                nc.vector.tensor_copy(dcol[:], ppre[:, HID + 1:HID + 2])
                nc.sync.dma_start(d_dram[b * 128:(b + 1) * 128, :], dcol[:])
                d64 = wp.tile([128, 64], F32, tag="d64")
                nc.vector.tensor_copy(d64[:], dcol[:].to_broadcast([128, 64]))
                nc.sync.dma_start(drep[b * 128:(b + 1) * 128, :], d64[:])
                nrow = min(128, cfg.V - b * 128)
                nc.sync.dma_start(shard[b * 128:b * 128 + nrow, :],
                                  rows[0:nrow, :])

            def prep_layer1():
                for b in range(cfg.NB):
                    ppre = pp.tile([128, HID + 2], F32, tag="mm")
                    for c in range(cfg.N_IN // 128):
                        xt = wp.tile([128, 128], F32, tag="xt")
                        nc.sync.dma_start(
                            xt[:],
                            xT[c * 128:(c + 1) * 128,
                               b * 128:(b + 1) * 128])
                        nc.tensor.matmul(ppre[:], lhsT=xt[:], rhs=rhs1[c][:],
                                         start=(c == 0),
                                         stop=(c == cfg.N_IN // 128 - 1))
                    build_rows(ppre, b, shards[0], d_drams[0], dreps[0])

            def prep_layer2():
                for b in range(cfg.NB):
                    tp = pp.tile([128, 128], F32, tag="mm")
                    nc.tensor.transpose(
                        tp[:], h2st[:, b * HID:(b + 1) * HID], ident[:])
                    h2T = wp.tile([128, 128], F32, tag="h2T")
                    nc.scalar.activation(h2T[:], tp[:], AF.Copy)
                    ppre = pp.tile([128, HID + 2], F32, tag="mm")
                    nc.tensor.matmul(ppre[:], lhsT=h2T[:], rhs=rhs2[0][:],
                                     start=True, stop=True)
                    build_rows(ppre, b, shards[1], d_drams[1], dreps[1])

            def allgather(li):
                nc.gpsimd.collective_compute(
                    "AllGather", mybir.AluOpType.bypass,
                    replica_groups=groups,
                    ins=[shards[li][:, :].opt()],
                    outs=[tables[li][0:cfg.N, :].opt()],
                )
                nc.sync.dma_start(tables[li][cfg.N:cfg.N + 1, :],
                                  sent_d[:, :])

            # ======== main per-layer loop ========
            def main_layer(li, epilogue):
                table, d_dram = tables[li], d_drams[li]
                dgd, dsd = dgds[li], dsds[li]
                drep = dreps[li]
                PB = cfg.PIECE_BLOCKS
                gpp16 = (PB * cfg.GB) // 16
                for pi, (b0, nb) in enumerate(cfg.pieces):
                    Tn = nb * cfg.TPB
                    NCh = nb * cfg.CPB
                    Gn = nb * cfg.GB
                    gbase = b0 * cfg.GB
                    # d per group: one bulk int16 dma_gather per piece
                    dg = wp.tile([128, PB * cfg.TPB * 64], F32, tag="dg")
                    dg3 = dg[:, 0:Tn * 64].rearrange(
                        "p (t e) -> p t e", e=64)
                    nidx = wp.tile([128, gpp16], mybir.dt.int16, tag="nidx")
                    nc.vector.tensor_copy(
                        nidx[:, 0:Gn // 16],
                        nog16sb[:, pi * gpp16:pi * gpp16 + Gn // 16])
                    nc.gpsimd.dma_gather(
                        out_ap=dg3,
                        in_ap=drep[:, :],
                        idxs_ap=nidx[:, 0:Gn // 16],
                        num_idxs=Gn,
                        num_idxs_reg=Gn,
                        elem_size=64,
                        single_packet=False)
                    nc.sync.dma_start(
                        dgd[gbase:gbase + Gn].rearrange("(t p) -> p t", p=128),
                        dg3[:, :, 0:1].rearrange("p t e -> p (t e)"))
                    nc.sync.dma_start(
                        dsd[b0 * cfg.CPB * 128:(b0 * cfg.CPB + NCh) * 128]
                        .rearrange("(bb c i l) -> bb c i l",
                                   bb=nb, c=cfg.CPB, i=8, l=16),
                        dgd[gbase:gbase + Gn]
                        .rearrange("(bb c l) -> bb c () l",
                                   bb=nb, c=cfg.CPB, l=16)
                        .to_broadcast([nb, cfg.CPB, 8, 16]))
                    dsl = wp.tile([128, PB * cfg.CPB], F32, tag="dsl")
                    nc.sync.dma_start(
                        dsl[:, 0:NCh],
                        dsd[b0 * cfg.CPB * 128:(b0 * cfg.CPB + NCh) * 128]
                        .rearrange("(j p) -> p j", p=128))
                    # feature gather: one 128-row indirect DMA per slot-chunk
                    # (skip the all-sentinel tail chunks of each block; their
                    # junk contributions are zero-weighted at level 2)
                    gt = gtA if pi % 2 == 0 else gtB
                    for j in range(NCh):
                        if (j % cfg.CPB) >= used_chunks[b0 + j // cfg.CPB]:
                            continue
                        nc.gpsimd.indirect_dma_start(
                            out=gt[:, ROW * j:ROW * (j + 1)], out_offset=None,
                            in_=table[:, :],
                            in_offset=IndirectOffsetOnAxis(
                                ap=srcsb[:, b0 * cfg.CPB + j:
                                         b0 * cfg.CPB + j + 1],
                                axis=0))
                    gv = gt[:, 0:ROW * NCh].rearrange(
                        "p (j c) -> p j c", c=ROW)
                    # phase A: ex = exp(leaky_relu(s_src + d_dst))
                    sf = wp.tile([128, PB * cfg.CPB], F32, tag="sf")
                    sfv = sf[:, 0:NCh].rearrange("p j -> p j ()")
                    nc.vector.tensor_tensor(
                        sfv, gv[:, :, HID + 1:HID + 2],
                        gv[:, :, HID + 2:HID + 3], op=OP.add)
                    ep = wp.tile([128, PB * cfg.CPB], F32, tag="ep")
                    nc.vector.tensor_tensor(ep[:, 0:NCh], sf[:, 0:NCh],
                                            dsl[:, 0:NCh], op=OP.add)
                    es = wp.tile([128, PB * cfg.CPB], F32, tag="es")
                    nc.vector.tensor_scalar_mul(es[:, 0:NCh], ep[:, 0:NCh],
                                                0.2)
                    el = wp.tile([128, PB * cfg.CPB], F32, tag="el")
                    nc.vector.tensor_tensor(el[:, 0:NCh], ep[:, 0:NCh],
                                            es[:, 0:NCh], op=OP.max)
                    exf = wp.tile([128, PB * cfg.CPB], F32, tag="exf")
                    nc.scalar.activation(exf[:, 0:NCh], el[:, 0:NCh], AF.Exp)
                    exb = wp.tile([128, PB * cfg.CPB], BF, tag="exb")
                    nc.vector.tensor_copy(exb[:, 0:NCh], exf[:, 0:NCh])
                    # selector build: even chunks -> cols [48a, 48a+16),
                    # odd chunks -> cols [48a+32, 48a+48)
                    sel1 = selA if (b0 // cfg.PIECE_BLOCKS) % 2 == 0 else selB
                    npair = NCh // 2
                    exv = exb[:, 0:NCh].rearrange("p (a u) -> p a u", u=2)
                    maskv = mask16[:].rearrange("p l -> p () l") \
                        .to_broadcast([128, npair, 16])
                    selv = sel1[:, 0:48 * npair].rearrange(
                        "p (a w) -> p a w", w=48)
                    nc.vector.tensor_tensor(
                        selv[:, :, 0:16],
                        exv[:, :, 0:1].to_broadcast([128, npair, 16]),
                        maskv, op=OP.mult)
                    nc.vector.tensor_tensor(
                        selv[:, :, 32:48],
                        exv[:, :, 1:2].to_broadcast([128, npair, 16]),
                        maskv, op=OP.mult)
                    # level 1 + level 2
                    for bb in range(nb):
                        b = b0 + bb
                        pl2 = pp.tile([128, RHSW], F32, tag="l2")
                        for t in range(cfg.TPB):
                            pl1 = pp.tile([128, RHSW], F32, tag="l1")
                            for al in range(4):
                                for u in range(2):
                                    j = bb * cfg.CPB + t * 8 + 2 * al + u
                                    A = j // 2
                                    nc.tensor.matmul(
                                        pl1[32 * al:32 * al + 32, :],
                                        lhsT=sel1[:, 48 * A + 16 * u:
                                                  48 * A + 16 * u + 32],
                                        rhs=gt[:, ROW * j:ROW * j + RHSW],
                                        start=(u == 0), stop=(u == 1),
                                        tile_position=(0, 32 * al))
                            gs = sp.tile([128, RHSW], F32, tag="gsum")
                            nc.scalar.activation(gs[:], pl1[:], AF.Copy)
                            T = b * cfg.TPB + t
                            l2s = wp.tile([128, 128], F32, tag="l2s")
                            nc.vector.tensor_tensor(
                                l2s[:],
                                glabsb[:, T:T + 1].to_broadcast([128, 128]),
                                iota2[:], op=OP.is_equal)
                            nc.tensor.matmul(pl2[:], lhsT=l2s[:], rhs=gs[:],
                                             start=(t == 0),
                                             stop=(t == cfg.TPB - 1))
                        epilogue(b, pl2)

            def epi_norm(pl2, brow):
                den = wp.tile([128, 1], F32, tag="den")
                nc.vector.tensor_scalar_max(den[:], pl2[:, HID:HID + 1],
                                            1e-30)
                rec = wp.tile([128, 1], F32, tag="rec")
                nc.vector.reciprocal(rec[:], den[:])
                hb = wp.tile([128, HID], F32, tag="hb")
                nc.vector.tensor_scalar_mul(hb[:], pl2[:, 0:HID],
                                            rec[:, 0:1])
                nc.vector.tensor_tensor(hb[:], hb[:], brow[:], op=OP.add)
                return hb

            def epilogue1(b, pl2):
                hb = epi_norm(pl2, b1r)
                nc.scalar.activation(h2st[:, b * HID:(b + 1) * HID], hb[:],
                                     AF.Relu)

            def epilogue2(b, pl2):
                hb = epi_norm(pl2, b2r)
                h3 = wp.tile([128, HID], F32, tag="h3")
                nc.scalar.activation(h3[:], hb[:], AF.Relu)
                tp = pp.tile([128, 128], F32, tag="mm")
                nc.tensor.transpose(tp[:], h3[:], ident[:])
                h3T = wp.tile([128, 128], F32, tag="h3T")
                nc.scalar.activation(h3T[:], tp[:], AF.Copy)
                po = pp.tile([128, NCLS], F32, tag="mm")
                nc.tensor.matmul(po[:], lhsT=h3T[:], rhs=wosb[:],
                                 start=True, stop=True)
                lg = wp.tile([128, NCLS], F32, tag="lg")
                nc.vector.tensor_tensor(lg[:], po[:], bor[:], op=OP.add)
                mx = wp.tile([128, 1], F32, tag="mx")
                nc.vector.tensor_reduce(mx[:], lg[:],
                                        axis=mybir.AxisListType.X, op=OP.max)
                lgs = wp.tile([128, NCLS], F32, tag="lgs")
                nc.vector.tensor_scalar(lgs[:], lg[:], mx[:, 0:1], None,
                                        op0=OP.subtract)
                pe = wp.tile([128, NCLS], F32, tag="pe")
                rs = wp.tile([128, 1], F32, tag="rs")
                nc.scalar.activation(pe[:], lgs[:], AF.Exp,
                                     accum_out=rs[:, 0:1])
                rr = wp.tile([128, 1], F32, tag="rr")
                nc.vector.reciprocal(rr[:], rs[:])
                ot = wp.tile([128, NCLS], F32, tag="ot")
                nc.vector.tensor_scalar_mul(ot[:], pe[:], rr[:, 0:1])
                nc.sync.dma_start(out_t[b * 128:(b + 1) * 128, :], ot[:])

            # ======== schedule ========
            stage = _stage
            if stage >= 2:
                prep_layer1()
            if stage >= 3:
                allgather(0)
            if stage >= 4:
                main_layer(0, epilogue1)
            if stage >= 5:
                prep_layer2()
                allgather(1)
            if stage >= 6:
                main_layer(1, epilogue2)
            if stage < 6:
                # touch the output so the program remains well formed
                zz = wp.tile([128, NCLS], F32, tag="zz")
                nc.gpsimd.memset(zz[:], 0.0)
                for b in range(cfg.NB):
                    nc.sync.dma_start(out_t[b * 128:(b + 1) * 128, :], zz[:])

    nc._gat_dbg = {
        "table0": tables[0].tensor.name, "table1": tables[1].tensor.name,
        "shard0": shards[0].tensor.name, "shard1": shards[1].tensor.name,
        "ddram0": d_drams[0].tensor.name, "ddram1": d_drams[1].tensor.name,
        "dgd0": dgds[0].tensor.name, "dsd0": dsds[0].tensor.name,
    }
    nc.compile()
    return nc


# ------------------------------------------------------------------ run ---
_PROG_CACHE = {}


def _get_program(cfg, used_chunks=None):
    key = (cfg.N, cfg.E, cfg.NCORES, used_chunks)
    if key not in _PROG_CACHE:
        _PROG_CACHE[key] = build_program(cfg, used_chunks)
    return _PROG_CACHE[key]


_PRE_CACHE = {}


def run(cfg, inputs, trace=False):
    from concourse.bass_utils import run_bass_kernel_spmd

    ei = np.asarray(inputs["edge_index"])
    pkey = (cfg.N, cfg.E, int(ei[0, :16].sum()), int(ei[1, -16:].sum()))
    if pkey not in _PRE_CACHE:
        _PRE_CACHE[pkey] = preprocess(cfg, ei)
    pre = _PRE_CACHE[pkey]
    nc = _get_program(cfg, _used_chunks(cfg, pre))
    in_maps = _make_in_maps(cfg, inputs, pre)
    res = run_bass_kernel_spmd(nc, in_maps, core_ids=list(range(cfg.NCORES)),
                               trace=trace)
    outs = [res.results[k]["out"][:cfg.V] for k in range(cfg.NCORES)]
    full = np.concatenate(outs, axis=0).astype(np.float32)
    return full, res


def timed_run(cfg, inputs, iters=3):
    """Time device execution with device-resident inputs (axon transfer
    excluded). Returns (best_seconds, outputs_of_last_iter)."""
    import time

    import jax
    from jax.sharding import Mesh, PartitionSpec
    from jax.experimental.shard_map import shard_map
    import concourse.mybir as mybir
    from concourse import bass2jax
    from concourse.bass2jax import _bass_exec_p, partition_id_tensor

    bass2jax.install_neuronx_cc_hook()
    ei = np.asarray(inputs["edge_index"])
    pkey = (cfg.N, cfg.E, int(ei[0, :16].sum()), int(ei[1, -16:].sum()))
    if pkey not in _PRE_CACHE:
        _PRE_CACHE[pkey] = preprocess(cfg, ei)
    pre = _PRE_CACHE[pkey]
    nc = _get_program(cfg, _used_chunks(cfg, pre))
    in_maps = _make_in_maps(cfg, inputs, pre)

    partition_name = (nc.partition_id_tensor.name
                      if nc.partition_id_tensor else None)
    in_names, out_names, out_avals, zero_outs = [], [], [], []
    for alloc in nc.m.functions[0].allocations:
        if not isinstance(alloc, mybir.MemoryLocationSet):
            continue
        name = alloc.memorylocations[0].name
        if alloc.kind == "ExternalInput":
            if name != partition_name:
                in_names.append(name)
        elif alloc.kind == "ExternalOutput":
            out_names.append(name)
            shape = tuple(alloc.tensor_shape)
            dtype = mybir.dt.np(alloc.dtype)
            out_avals.append(jax.core.ShapedArray(shape, dtype))
            zero_outs.append(np.zeros(shape, dtype))
    n_params = len(in_names)
    n_outs = len(out_avals)
    all_in_names = list(in_names) + list(out_names)
    if partition_name is not None:
        all_in_names.append(partition_name)
    donate = tuple(range(n_params, n_params + n_outs))

    def _body(*args):
        operands = list(args)
        if partition_name is not None:
            operands.append(partition_id_tensor())
        outs = _bass_exec_p.bind(
            *operands, out_avals=tuple(out_avals),
            in_names=tuple(all_in_names), out_names=tuple(out_names),
            lowering_input_output_aliases=(),
            sim_require_finite=True, sim_require_nnan=True, nc=nc)
        return tuple(outs)

    devices = jax.devices()[:cfg.NCORES]
    mesh = Mesh(np.asarray(devices), ("core",))
    in_specs = (PartitionSpec("core"),) * (n_params + n_outs)
    out_specs = (PartitionSpec("core"),) * len(out_names)
    sharded = jax.jit(
        shard_map(_body, mesh=mesh, in_specs=in_specs, out_specs=out_specs,
                  check_rep=False),
        donate_argnums=donate, keep_unused=True)
    concat_in = [
        np.concatenate([np.asarray(in_maps[c][nm]) for c in range(cfg.NCORES)],
                       axis=0)
        for nm in in_names]
    sharding = jax.sharding.NamedSharding(mesh, PartitionSpec("core"))
    dev_in = [jax.device_put(a, sharding) for a in concat_in]
    times = []
    out_arrs = None
    for _ in range(iters):
        zo = [jax.device_put(
            np.zeros((cfg.NCORES * z.shape[0], *z.shape[1:]), z.dtype),
            sharding) for z in zero_outs]
        jax.block_until_ready(zo)
        t0 = time.time()
        out_arrs = sharded(*dev_in, *zo)
        jax.block_until_ready(out_arrs)
        times.append(time.time() - t0)
    oi = out_names.index("out")
    full = np.asarray(out_arrs[oi]).reshape(cfg.NCORES, cfg.VPAD, cfg.NCLS)
    out = np.concatenate([full[c, :cfg.V] for c in range(cfg.NCORES)], axis=0)
    return min(times), out.astype(np.float32)


def _make_in_maps(cfg, inputs, pre):
    x = np.asarray(inputs["x"], np.float32)
    common = {
        "W1": np.asarray(inputs["W1"], np.float32),
        "W2": np.asarray(inputs["W2"], np.float32),
        "Wo": np.asarray(inputs["Wo"], np.float32),
        "a1s": np.asarray(inputs["a1_src"], np.float32),
        "a1d": np.asarray(inputs["a1_dst"], np.float32),
        "a2s": np.asarray(inputs["a2_src"], np.float32),
        "a2d": np.asarray(inputs["a2_dst"], np.float32),
        "b1": np.asarray(inputs["b1"], np.float32),
        "b2": np.asarray(inputs["b2"], np.float32),
        "bo": np.asarray(inputs["bo"], np.float32),
    }
    in_maps = []
    for k in range(cfg.NCORES):
        xs = x[k * cfg.V:(k + 1) * cfg.V]
        xT = np.zeros((cfg.N_IN, cfg.VPAD), np.float32)
        xT[:, :cfg.V] = xs.T
        m = dict(common)
        m["xT"] = np.ascontiguousarray(xT)
        m["src_slot"] = pre[k]["src_slot"]
        m["nog"] = pre[k]["nog"]
        m["nog16"] = pre[k]["nog16"]
        m["glabel"] = pre[k]["glabel"]
        in_maps.append(m)
    return in_maps


def kernel(**inputs):
    cfg = DEFAULT_CFG
    full, _ = run(cfg, inputs, trace=False)
    return full
